# revision 4
# baseline (speedup 1.0000x reference)
"""GQA kernel for 8 trn2 NeuronCores.

Sharding: tensor-parallel over heads. Core c owns KV head c and Q heads
4c..4c+3 (q-dim cols 256c:256c+256 of Wq, col 64c:64c+64 of Wk/Wv, rows
256c:256c+256 of Wo). Each core computes a partial output [B,S,E]
(its ctx slice @ its Wo row-slice); host sums the 8 partials.

Device algorithm (per core) — v3, software-pipelined across batches:
  A1. Q.T = (Wq/8).T @ X.T as two head-PAIR tiles [128, S] (Wq prescaled
      on host so the PSUM->SBUF eviction is a plain copy).
  A2. K.T [64, S] (dup to 128 partitions) and V natural [S, 64+ones]
      via per-512-col accumulation groups (K rows 0:64, V rows 64:128
      col-packed PSUM) + DMA transposes for V.
  B.  16 units per batch = (q-head h, 512-wide q block j). Per unit,
      a kv-chunk pipeline: sc = K.T_chunk.T @ Q.T slice -> PSUM [128,512]
      (1 bank, 3 bufs), exp on ScalarE -> pt bf16 SBUF, ctx.T[0:65]
      += V_aug.T @ pt accumulated in PSUM [128,512] (rows 0:64 ctx,
      row 64 softmax denominator). ScalarE paces the loop; the PE deficit
      is filled with interleaved matmuls from A1/A2 of the next batch and
      phase C of the current/previous batch (filler streams).
      Normalize: DVE recip + gpsimd partition-broadcast + DVE mul into
      ctxT tiles [128, 512] per (head-pair, j).
  C.  out_partial[t*128:+128, :] = ctxT.T @ Wo_c, 2-chunk accumulation,
      evicted bf16 and DMA'd out per 128-row tile.

All matmuls bf16 inputs / fp32 PSUM. PSUM banks: sc(3) + ctx(2) + acc(3).
"""

import numpy as np
import ml_dtypes
from collections import deque

B = 2
S = 2048
E = 2048
HD = 64          # head dim
HPC = 4          # q heads per core
QD = HPC * HD    # 256 per-core q dims
NCORES = 8
EC = E // 128    # 16 contraction chunks
NKV = S // 128   # 16 kv chunks of 128
NJ = S // 512    # 4 q blocks of 512
BF16 = ml_dtypes.bfloat16

_cache = {}


def _build():
    from contextlib import ExitStack
    from concourse import bacc, tile
    import concourse.mybir as mybir

    bf16 = mybir.dt.bfloat16
    f32 = mybir.dt.float32
    EXP = mybir.ActivationFunctionType.Exp

    nc = bacc.Bacc(
        "TRN2", target_bir_lowering=False, debug=False, num_devices=NCORES)
    qT_d = nc.declare_dram_parameter("qT", [B, E, S], bf16, isOutput=False)
    kT_d = nc.declare_dram_parameter("kT", [B, E, S], bf16, isOutput=False)
    vT_d = nc.declare_dram_parameter("vT", [B, E, S], bf16, isOutput=False)
    wq_d = nc.declare_dram_parameter("wq", [E, QD], bf16, isOutput=False)
    wk_d = nc.declare_dram_parameter("wk", [E, HD], bf16, isOutput=False)
    wv_d = nc.declare_dram_parameter("wv", [E, HD], bf16, isOutput=False)
    wo_d = nc.declare_dram_parameter("wo", [QD, E], bf16, isOutput=False)
    out_d = nc.declare_dram_parameter("out", [B, S, E], bf16, isOutput=True)

    qT_r = qT_d.rearrange("b (c p) s -> b p c s", p=128)   # [B,128,16,S]
    kT_r = kT_d.rearrange("b (c p) s -> b p c s", p=128)
    vT_r = vT_d.rearrange("b (c p) s -> b p c s", p=128)

    with ExitStack() as ctx:
        tc = ctx.enter_context(tile.TileContext(nc))
        # ---- pools ----
        wpool = ctx.enter_context(tc.tile_pool(name="w", bufs=1))
        qin = ctx.enter_context(tc.tile_pool(name="qin", bufs=4))
        kin = ctx.enter_context(tc.tile_pool(name="kin", bufs=5))
        vin = ctx.enter_context(tc.tile_pool(name="vin", bufs=5))
        qts = ctx.enter_context(tc.tile_pool(name="qts", bufs=4))
        ktp = ctx.enter_context(tc.tile_pool(name="ktp", bufs=2))
        vtp = ctx.enter_context(tc.tile_pool(name="vtp", bufs=2))
        vnp = ctx.enter_context(tc.tile_pool(name="vnp", bufs=1))
        ptp = ctx.enter_context(tc.tile_pool(name="ptp", bufs=3))
        ctp = ctx.enter_context(tc.tile_pool(name="ctp", bufs=1))
        ostp = ctx.enter_context(tc.tile_pool(name="ostp", bufs=3))
        smp = ctx.enter_context(tc.tile_pool(name="smp", bufs=2))
        scp = ctx.enter_context(tc.tile_pool(name="scp", bufs=3, space="PSUM"))
        cxp = ctx.enter_context(tc.tile_pool(name="cxp", bufs=2, space="PSUM"))
        accp = ctx.enter_context(tc.tile_pool(name="accp", bufs=3,
                                              space="PSUM"))

        # ---- weights (loaded once) ----
        wq_sb = wpool.tile([128, EC, QD], bf16)
        nc.sync.dma_start(wq_sb[:], wq_d.rearrange("(c p) m -> p c m", p=128))
        wk_sb = wpool.tile([128, EC, HD], bf16)
        nc.sync.dma_start(wk_sb[:], wk_d.rearrange("(c p) m -> p c m", p=128))
        wv_sb = wpool.tile([128, EC, HD], bf16)
        nc.sync.dma_start(wv_sb[:], wv_d.rearrange("(c p) m -> p c m", p=128))
        wo_sb = wpool.tile([128, 2, E], bf16)
        nc.sync.dma_start(wo_sb[:], wo_d.rearrange("(c p) e -> p c e", p=128))

        # V_aug tiles, ones column set once (transposes only write 0:HD)
        vn_tiles = [[vnp.tile([128, HD + 1], bf16, name=f"vn{b}_{c}")
                     for c in range(NKV)] for b in range(B)]
        for b in range(B):
            for c in range(NKV):
                nc.vector.memset(vn_tiles[b][c][:, HD:HD + 1], 1.0)

        # per-batch persistent tiles
        qp_sb = {}    # (b, pair) -> Q.T pair tile [128, S]
        kt2_sb = {}   # b -> K.T dup tile [128, S]
        ctxT = {}     # (b, pair, j) -> normalized ctx.T tile [128, 512]

        def load_q(b):
            quads = []
            for g in range(4):
                qt = qin.tile([128, 4, S], bf16, tag="qt", name=f"q{b}_{g}")
                nc.sync.dma_start(qt[:], qT_r[b, :, 4 * g:4 * g + 4, :])
                quads.append(qt)
            return quads

        def a1_stream(b, quads):
            """Q.T projection: 8 groups of 16 accumulating matmuls."""
            qp_sb[(b, 0)] = qts.tile([128, S], bf16, tag="qp", name=f"qp{b}0")
            qp_sb[(b, 1)] = qts.tile([128, S], bf16, tag="qp", name=f"qp{b}1")
            for m in range(2):
                for t in range(NJ):
                    acc = accp.tile([128, 512], f32, tag="acc", name="a1acc")
                    for e in range(EC):
                        nc.tensor.matmul(
                            acc[:], lhsT=wq_sb[:, e, m * 128:(m + 1) * 128],
                            rhs=quads[e // 4][:, e % 4, t * 512:(t + 1) * 512],
                            start=(e == 0), stop=(e == EC - 1))
                        yield
                    nc.vector.tensor_copy(
                        qp_sb[(b, m)][:, t * 512:(t + 1) * 512], acc[:])

        def a2_stream(b):
            """K.T / V projections, col block (ti) at a time. Issues its own
            input DMAs (per-ti quads) and V transposes."""
            kt2 = ktp.tile([128, S], bf16, tag="kt2", name=f"kt2_{b}")
            kt2_sb[b] = kt2
            for ti in range(NJ):
                kq = []
                for g in range(4):
                    kt_in = kin.tile([128, 4, 512], bf16, tag="ki",
                                     name=f"k{b}_{ti}_{g}")
                    nc.sync.dma_start(
                        kt_in[:],
                        kT_r[b, :, 4 * g:4 * g + 4, ti * 512:(ti + 1) * 512])
                    vt_in = vin.tile([128, 4, 512], bf16, tag="vi",
                                     name=f"v{b}_{ti}_{g}")
                    nc.sync.dma_start(
                        vt_in[:],
                        vT_r[b, :, 4 * g:4 * g + 4, ti * 512:(ti + 1) * 512])
                    kq.append((kt_in, vt_in))
                acc = accp.tile([128, 512], f32, tag="acc", name="a2acc")
                for e in range(EC):
                    kt_in, vt_in = kq[e // 4]
                    nc.tensor.matmul(
                        acc[0:64, :], lhsT=wk_sb[:, e, :],
                        rhs=kt_in[:, e % 4, :],
                        start=(e == 0), stop=(e == EC - 1))
                    yield
                    nc.tensor.matmul(
                        acc[64:128, :], lhsT=wv_sb[:, e, :],
                        rhs=vt_in[:, e % 4, :],
                        start=(e == 0), stop=(e == EC - 1),
                        tile_position=(0, 64))
                    yield
                nc.vector.tensor_copy(
                    kt2[0:64, ti * 512:(ti + 1) * 512], acc[0:64, :])
                vt = vtp.tile([64, 512], bf16, tag="vt", name=f"vt{b}_{ti}")
                nc.vector.tensor_copy(vt[:], acc[64:128, :])
                for c in range(4):
                    nc.sync.dma_start_transpose(
                        out=vn_tiles[b][ti * 4 + c][:, 0:HD],
                        in_=vt[:, c * 128:(c + 1) * 128])
            nc.sync.dma_start(kt2[64:128, :], kt2[0:64, :])

        def c_stream(b, j):
            """Phase C for output rows [j*512, (j+1)*512): 4 row-tiles."""
            for tt in range(4):
                t = j * 4 + tt
                ost = ostp.tile([128, E], bf16, tag="ost", name=f"o{b}_{t}")
                for e in range(4):
                    acc = accp.tile([128, 512], f32, tag="acc", name="cacc")
                    for kc in range(2):
                        nc.tensor.matmul(
                            acc[:],
                            lhsT=ctxT[(b, kc, j)][:, tt * 128:(tt + 1) * 128],
                            rhs=wo_sb[:, kc, e * 512:(e + 1) * 512],
                            start=(kc == 0), stop=(kc == 1))
                        yield
                    nc.vector.tensor_copy(ost[:, e * 512:(e + 1) * 512],
                                          acc[:])
                nc.sync.dma_start(out_d[b, t * 128:(t + 1) * 128, :], ost[:])

        # ---- filler machinery ----
        streams = deque()

        def pump(n):
            done = 0
            while done < n and streams:
                try:
                    next(streams[0])
                    done += 1
                except StopIteration:
                    streams.popleft()

        def drain():
            while streams:
                try:
                    next(streams[0])
                except StopIteration:
                    streams.popleft()

        def b_unit(b, j, h):
            """Attention for q-head h on q rows [j*512, (j+1)*512)."""
            kc, off = h // 2, (h % 2) * 64
            kt2 = kt2_sb[b]
            qp = qp_sb[(b, kc)]
            ctx_ps = cxp.tile([128, 512], f32, tag="ctx", name=f"cx{b}{j}{h}")
            pts = {}
            for kv in range(NKV):
                sc = scp.tile([128, 512], f32, tag="sc", name="sc")
                nc.tensor.matmul(
                    sc[:], lhsT=kt2[off:off + 64, kv * 128:(kv + 1) * 128],
                    rhs=qp[off:off + 64, j * 512:(j + 1) * 512],
                    start=True, stop=True)
                pt = ptp.tile([128, 512], bf16, tag="pt", name="pt")
                nc.scalar.activation(pt[:], sc[:], EXP)
                pts[kv] = pt
                if kv >= 2:
                    nc.tensor.matmul(
                        ctx_ps[0:HD + 1, :],
                        lhsT=vn_tiles[b][kv - 2][:, 0:HD + 1],
                        rhs=pts.pop(kv - 2)[:],
                        start=(kv - 2 == 0), stop=False)
                pump(2)
            for kv in (NKV - 2, NKV - 1):
                nc.tensor.matmul(
                    ctx_ps[0:HD + 1, :],
                    lhsT=vn_tiles[b][kv][:, 0:HD + 1],
                    rhs=pts.pop(kv)[:],
                    start=False, stop=(kv == NKV - 1))
            # normalize by softmax denominator (row HD)
            recip = smp.tile([1, 512], f32, tag="recip", name="recip")
            nc.vector.reciprocal(recip[:], ctx_ps[HD:HD + 1, :])
            rb = smp.tile([64, 512], f32, tag="rb", name="rb")
            nc.gpsimd.partition_broadcast(rb[:], recip[:])
            nc.vector.tensor_mul(
                ctxT[(b, kc, j)][off:off + 64, :], ctx_ps[0:64, :], rb[:])
            pump(4)

        # ================= schedule =================
        # prolog: batch 0 A phase
        q0 = load_q(0)
        for g in (a1_stream(0, q0), a2_stream(0)):
            streams.append(g)
        drain()

        for b in range(B):
            for kc in range(2):
                for j in range(NJ):
                    ctxT[(b, kc, j)] = ctp.tile(
                        [128, 512], bf16, name=f"ctxT{b}_{kc}_{j}")
            if b + 1 < B:
                qn = load_q(b + 1)
                streams.append(a1_stream(b + 1, qn))
                streams.append(a2_stream(b + 1))
            for j in range(NJ):
                for h in range(HPC):
                    b_unit(b, j, h)
                streams.append(c_stream(b, j))
        drain()

    nc.compile()
    return nc


def _get_nc():
    if "nc" not in _cache:
        _cache["nc"] = _build()
    return _cache["nc"]


def kernel(query, key, value, Wq, Wk, Wv, Wo, _trace=False):
    from concourse.bass_utils import run_bass_kernel_spmd

    def t_bf16(x):
        return np.ascontiguousarray(
            np.asarray(x, np.float32).astype(BF16).transpose(0, 2, 1))

    qT = t_bf16(query)
    kT = t_bf16(key)
    vT = t_bf16(value)
    # prescale Wq by the 1/sqrt(HD) attention scale (exact power of 2)
    Wq = (np.asarray(Wq, np.float32) * 0.125).astype(BF16)
    Wk = np.asarray(Wk, np.float32).astype(BF16)
    Wv = np.asarray(Wv, np.float32).astype(BF16)
    Wo = np.asarray(Wo, np.float32).astype(BF16)

    in_maps = []
    for c in range(NCORES):
        in_maps.append({
            "qT": qT, "kT": kT, "vT": vT,
            "wq": np.ascontiguousarray(Wq[:, c * QD:(c + 1) * QD]),
            "wk": np.ascontiguousarray(Wk[:, c * HD:(c + 1) * HD]),
            "wv": np.ascontiguousarray(Wv[:, c * HD:(c + 1) * HD]),
            "wo": np.ascontiguousarray(Wo[c * QD:(c + 1) * QD, :]),
        })

    nc = _get_nc()
    res = run_bass_kernel_spmd(nc, in_maps, list(range(NCORES)), trace=_trace)
    out = res.results[0]["out"].astype(np.float32)
    for c in range(1, NCORES):
        out += res.results[c]["out"].astype(np.float32)
    if _trace:
        _cache["last_exec_time_ns"] = res.exec_time_ns
        _cache["last_results"] = res
    return out


# revision 13
# speedup vs baseline: 1.0621x; 1.0621x over previous
"""GQA kernel for 8 trn2 NeuronCores.

Sharding: tensor-parallel over heads. Core c owns KV head c and Q heads
4c..4c+3 (q-dim cols 256c:256c+256 of Wq, col 64c:64c+64 of Wk/Wv, rows
256c:256c+256 of Wo). Each core computes a partial output [B,S,E]
(its ctx slice @ its Wo row-slice); host sums the 8 partials.

Device algorithm (per core) — v4, software-pipelined across batches:
  A1. Q.T = (Wq/8).T @ X.T as two head-PAIR tiles [128, S] (Wq prescaled
      on host so the PSUM->SBUF eviction is a plain copy).
  A2. K.T [64, S] (dup to 128 partitions) and V natural [S, 64+ones]
      via per-512-col accumulation groups (K rows 0:64, V rows 64:128
      col-packed PSUM) + DMA transposes for V.
  B.  16 units per batch = (q-head h, 512-wide q block j). Per unit,
      a kv-PAIR pipeline (8 steps): two score matmuls into a [128,1024]
      PSUM tile (2 banks), one exp on ScalarE -> pt bf16 [128,1024],
      two ctx matmuls (lagged one pair) accumulating ctx.T[0:65]
      (row 64 = softmax denominator, via ones column in V_aug).
      ScalarE paces the loop; the PE deficit is filled with matmuls from
      A1/A2 of the next batch and phase C of the current batch, pumped
      from filler streams gated by an emission-time DMA-arrival model
      (a filler that would wait on DMA would head-of-line block the
      in-order PE queue).
      Normalize: DVE recip + gpsimd partition-broadcast + DVE mul into
      ctxT tiles [128, 512] per (head-pair, j).
  C.  out_partial[t*128:+128, :] = ctxT.T @ Wo_c, 2-chunk accumulation,
      evicted bf16 and DMA'd out per 128-row tile.

All matmuls bf16 inputs / fp32 PSUM. PSUM banks: sc(2x2) + ctx(2) +
acc(2).
"""

import numpy as np
import ml_dtypes
from collections import deque

B = 2
S = 2048
E = 2048
HD = 64          # head dim
HPC = 4          # q heads per core
QD = HPC * HD    # 256 per-core q dims
NCORES = 8
EC = E // 128    # 16 contraction chunks
NKV = S // 128   # 16 kv chunks of 128
NP = NKV // 2    # 8 kv pairs
NJ = S // 512    # 4 q blocks of 512
BF16 = ml_dtypes.bfloat16

NOTREADY = object()

_cache = {}


def _build():
    from contextlib import ExitStack
    from concourse import bacc, tile
    import concourse.mybir as mybir

    bf16 = mybir.dt.bfloat16
    f32 = mybir.dt.float32
    EXP = mybir.ActivationFunctionType.Exp

    nc = bacc.Bacc(
        "TRN2", target_bir_lowering=False, debug=False, num_devices=NCORES)
    qT_d = nc.declare_dram_parameter("qT", [B, E, S], bf16, isOutput=False)
    kT_d = nc.declare_dram_parameter("kT", [B, E, S], bf16, isOutput=False)
    vT_d = nc.declare_dram_parameter("vT", [B, E, S], bf16, isOutput=False)
    wq_d = nc.declare_dram_parameter("wq", [E, QD], bf16, isOutput=False)
    wk_d = nc.declare_dram_parameter("wk", [E, HD], bf16, isOutput=False)
    wv_d = nc.declare_dram_parameter("wv", [E, HD], bf16, isOutput=False)
    wo_d = nc.declare_dram_parameter("wo", [QD, E], bf16, isOutput=False)
    out_d = nc.declare_dram_parameter("out", [B, S, E], bf16, isOutput=True)

    qT_r = qT_d.rearrange("b (c p) s -> b p c s", p=128)   # [B,128,16,S]
    kT_r = kT_d.rearrange("b (c p) s -> b p c s", p=128)
    vT_r = vT_d.rearrange("b (c p) s -> b p c s", p=128)

    class Sched:
        """Emission-time clock model: pe = estimated wall when the
        instruction being emitted will run; dma = when the DMA queue
        drains. Used only to gate filler emission, not for correctness."""
        NS_PER_BYTE = 1.0 / 360.0e9 * 1e9   # single shared DMA bus

        def __init__(self):
            self.pe = 0.0
            self.dma = 0.0

        def dma_issue(self, nbytes):
            self.dma = max(self.dma, self.pe) + 700 + nbytes * self.NS_PER_BYTE
            return self.dma

    sched = Sched()

    with ExitStack() as ctx:
        tc = ctx.enter_context(tile.TileContext(nc))
        # ---- pools ----
        wpool = ctx.enter_context(tc.tile_pool(name="w", bufs=1))
        qin = ctx.enter_context(tc.tile_pool(name="qin", bufs=2))
        kin = ctx.enter_context(tc.tile_pool(name="kin", bufs=3))
        vin = ctx.enter_context(tc.tile_pool(name="vin", bufs=3))
        qts = ctx.enter_context(tc.tile_pool(name="qts", bufs=4))
        ktp = ctx.enter_context(tc.tile_pool(name="ktp", bufs=2))
        vtp = ctx.enter_context(tc.tile_pool(name="vtp", bufs=2))
        vnp = ctx.enter_context(tc.tile_pool(name="vnp", bufs=1))
        ptp = ctx.enter_context(tc.tile_pool(name="ptp", bufs=3))
        ctp = ctx.enter_context(tc.tile_pool(name="ctp", bufs=1))
        ostp = ctx.enter_context(tc.tile_pool(name="ostp", bufs=2))
        smp = ctx.enter_context(tc.tile_pool(name="smp", bufs=2))
        scp = ctx.enter_context(tc.tile_pool(name="scp", bufs=2, space="PSUM"))
        cxp = ctx.enter_context(tc.tile_pool(name="cxp", bufs=2, space="PSUM"))
        accp = ctx.enter_context(tc.tile_pool(name="accp", bufs=2,
                                              space="PSUM"))

        # ---- weights: wq first, wo deferred (only phase C needs it) ----
        wq_sb = wpool.tile([128, EC, QD], bf16)
        nc.sync.dma_start(wq_sb[:], wq_d.rearrange("(c p) m -> p c m", p=128))
        sched.dma_issue(E * QD * 2)

        def load_q(b):
            quads = []
            for g in range(2):
                qt = qin.tile([128, 8, S], bf16, tag="qt", name=f"q{b}_{g}")
                nc.sync.dma_start(qt[:], qT_r[b, :, 8 * g:8 * g + 8, :])
                quads.append((qt, sched.dma_issue(128 * 8 * S * 2)))
            return quads

        q0 = load_q(0)

        wk_sb = wpool.tile([128, EC, HD], bf16)
        nc.sync.dma_start(wk_sb[:], wk_d.rearrange("(c p) m -> p c m", p=128))
        sched.dma_issue(E * HD * 2)
        wv_sb = wpool.tile([128, EC, HD], bf16)
        nc.sync.dma_start(wv_sb[:], wv_d.rearrange("(c p) m -> p c m", p=128))
        sched.dma_issue(E * HD * 2)
        wo_sb = wpool.tile([128, 2, E], bf16)

        # V_aug tiles, ones column set once (transposes only write 0:HD)
        vn_tiles = [[vnp.tile([128, HD + 1], bf16, name=f"vn{b}_{c}")
                     for c in range(NKV)] for b in range(B)]
        for b in range(B):
            for c in range(NKV):
                nc.vector.memset(vn_tiles[b][c][:, HD:HD + 1], 1.0)

        # per-batch persistent tiles
        qp_sb = {}    # (b, pair) -> Q.T pair tile [128, S]
        kt2_sb = {}   # b -> K.T dup tile [128, S]
        ctxT = {}     # (b, pair, j) -> normalized ctx.T tile [128, 512]

        def a1_stream(b, quads):
            """Q.T projection: 8 groups of 16 accumulating matmuls."""
            qp_sb[(b, 0)] = qts.tile([128, S], bf16, tag="qp", name=f"qp{b}0")
            qp_sb[(b, 1)] = qts.tile([128, S], bf16, tag="qp", name=f"qp{b}1")
            for m in range(2):
                for t in range(NJ):
                    acc = accp.tile([128, 512], f32, tag="acc", name="a1acc")
                    for e in range(EC):
                        qt, ready = quads[e // 8]
                        while sched.pe < ready:
                            yield NOTREADY, ready
                        nc.tensor.matmul(
                            acc[:], lhsT=wq_sb[:, e, m * 128:(m + 1) * 128],
                            rhs=qt[:, e % 8, t * 512:(t + 1) * 512],
                            start=(e == 0), stop=(e == EC - 1))
                        yield None, 0
                    nc.vector.tensor_copy(
                        qp_sb[(b, m)][:, t * 512:(t + 1) * 512], acc[:])

        def load_kv_ti(b, ti):
            quads = []
            for g in range(2):
                kt_in = kin.tile([128, 8, 512], bf16, tag="ki",
                                 name=f"k{b}_{ti}_{g}")
                nc.sync.dma_start(
                    kt_in[:],
                    kT_r[b, :, 8 * g:8 * g + 8, ti * 512:(ti + 1) * 512])
                sched.dma_issue(128 * 8 * 512 * 2)
                vt_in = vin.tile([128, 8, 512], bf16, tag="vi",
                                 name=f"v{b}_{ti}_{g}")
                nc.sync.dma_start(
                    vt_in[:],
                    vT_r[b, :, 8 * g:8 * g + 8, ti * 512:(ti + 1) * 512])
                quads.append((kt_in, vt_in,
                              sched.dma_issue(128 * 8 * 512 * 2)))
            return quads

        def a2_stream(b):
            """K.T / V projections, col block (ti) at a time, with its own
            prefetched input DMAs and V transposes."""
            kt2 = ktp.tile([128, S], bf16, tag="kt2", name=f"kt2_{b}")
            kt2_sb[b] = kt2
            pending = load_kv_ti(b, 0)
            for ti in range(NJ):
                kq = pending
                pending = load_kv_ti(b, ti + 1) if ti + 1 < NJ else None
                acc = accp.tile([128, 512], f32, tag="acc", name="a2acc")
                for e in range(EC):
                    kt_in, vt_in, ready = kq[e // 8]
                    while sched.pe < ready:
                        yield NOTREADY, ready
                    nc.tensor.matmul(
                        acc[0:64, :], lhsT=wk_sb[:, e, :],
                        rhs=kt_in[:, e % 8, :],
                        start=(e == 0), stop=(e == EC - 1))
                    yield None, 0
                    nc.tensor.matmul(
                        acc[64:128, :], lhsT=wv_sb[:, e, :],
                        rhs=vt_in[:, e % 8, :],
                        start=(e == 0), stop=(e == EC - 1),
                        tile_position=(0, 64))
                    yield None, 0
                nc.vector.tensor_copy(
                    kt2[0:64, ti * 512:(ti + 1) * 512], acc[0:64, :])
                vt = vtp.tile([64, 512], bf16, tag="vt", name=f"vt{b}_{ti}")
                nc.vector.tensor_copy(vt[:], acc[64:128, :])
                for c in range(4):
                    nc.sync.dma_start_transpose(
                        out=vn_tiles[b][ti * 4 + c][:, 0:HD],
                        in_=vt[:, c * 128:(c + 1) * 128])
            nc.sync.dma_start(kt2[64:128, :], kt2[0:64, :])

        def c_stream(b, j):
            """Phase C for output rows [j*512, (j+1)*512): 4 row-tiles."""
            for tt in range(4):
                t = j * 4 + tt
                ost = ostp.tile([128, E], bf16, tag="ost", name=f"o{b}_{t}")
                for e in range(4):
                    acc = accp.tile([128, 512], f32, tag="acc", name="cacc")
                    for kc in range(2):
                        nc.tensor.matmul(
                            acc[:],
                            lhsT=ctxT[(b, kc, j)][:, tt * 128:(tt + 1) * 128],
                            rhs=wo_sb[:, kc, e * 512:(e + 1) * 512],
                            start=(kc == 0), stop=(kc == 1))
                        yield None, 0
                    nc.vector.tensor_copy(ost[:, e * 512:(e + 1) * 512],
                                          acc[:])
                nc.sync.dma_start(out_d[b, t * 128:(t + 1) * 128, :], ost[:])
                sched.dma_issue(128 * E * 2)

        # ---- filler machinery ----
        streams = deque()

        def pump(n):
            done = tries = 0
            while done < n and streams and tries < len(streams):
                kind, ready = next(streams[0], ("END", 0))
                if kind is NOTREADY:
                    streams.rotate(-1)
                    tries += 1
                elif kind == "END":
                    streams.popleft()
                else:
                    done += 1
                    tries = 0

        def drain():
            while streams:
                stall = []
                progressed = False
                for _ in range(len(streams)):
                    kind, ready = next(streams[0], ("END", 0))
                    if kind is NOTREADY:
                        stall.append(ready)
                        streams.rotate(-1)
                    elif kind == "END":
                        streams.popleft()
                        progressed = True
                        break
                    else:
                        sched.pe += 213
                        progressed = True
                        break
                if not progressed and stall:
                    sched.pe = max(sched.pe, min(stall))

        def b_unit(b, j, h):
            """Attention for q-head h on q rows [j*512, (j+1)*512)."""
            kc, off = h // 2, (h % 2) * 64
            kt2 = kt2_sb[b]
            qp = qp_sb[(b, kc)]
            ctx_ps = cxp.tile([128, 512], f32, tag="ctx", name=f"cx{b}{j}{h}")
            prev = None
            for p in range(NP):
                sc = scp.tile([128, 1024], f32, tag="sc", name="sc")
                for half in range(2):
                    kv = 2 * p + half
                    nc.tensor.matmul(
                        sc[:, half * 512:(half + 1) * 512],
                        lhsT=kt2[off:off + 64, kv * 128:(kv + 1) * 128],
                        rhs=qp[off:off + 64, j * 512:(j + 1) * 512],
                        start=True, stop=True)
                pt = ptp.tile([128, 1024], bf16, tag="pt", name="pt")
                nc.scalar.activation(pt[:], sc[:], EXP)
                if prev is not None:
                    pp, ppt = prev
                    for half in range(2):
                        kv = 2 * pp + half
                        nc.tensor.matmul(
                            ctx_ps[0:HD + 1, :],
                            lhsT=vn_tiles[b][kv][:, 0:HD + 1],
                            rhs=ppt[:, half * 512:(half + 1) * 512],
                            start=(kv == 0), stop=False)
                prev = (p, pt)
                sched.pe += 1222
                pump(2)
            pp, ppt = prev
            for half in range(2):
                kv = 2 * pp + half
                nc.tensor.matmul(
                    ctx_ps[0:HD + 1, :],
                    lhsT=vn_tiles[b][kv][:, 0:HD + 1],
                    rhs=ppt[:, half * 512:(half + 1) * 512],
                    start=False, stop=(kv == NKV - 1))
            # normalize by softmax denominator (row HD)
            recip = smp.tile([1, 512], f32, tag="recip", name="recip")
            nc.vector.reciprocal(recip[:], ctx_ps[HD:HD + 1, :])
            rb = smp.tile([64, 512], f32, tag="rb", name="rb")
            nc.gpsimd.partition_broadcast(rb[:], recip[:])
            nc.vector.tensor_mul(
                ctxT[(b, kc, j)][off:off + 64, :], ctx_ps[0:64, :], rb[:])
            sched.pe += 2200
            pump(4)

        # ================= schedule =================
        # prolog: batch 0 A phase (DMA-paced)
        streams.append(a1_stream(0, q0))
        streams.append(a2_stream(0))
        drain()
        nc.sync.dma_start(wo_sb[:], wo_d.rearrange("(c p) e -> p c e", p=128))
        sched.dma_issue(QD * E * 2)

        for b in range(B):
            for kc in range(2):
                for j in range(NJ):
                    ctxT[(b, kc, j)] = ctp.tile(
                        [128, 512], bf16, name=f"ctxT{b}_{kc}_{j}")
            if b + 1 < B:
                qn = load_q(b + 1)
                streams.append(a1_stream(b + 1, qn))
                streams.append(a2_stream(b + 1))
            for j in range(NJ):
                for h in range(HPC):
                    b_unit(b, j, h)
                streams.append(c_stream(b, j))
        drain()

    nc.compile()
    return nc


def _get_nc():
    if "nc" not in _cache:
        _cache["nc"] = _build()
    return _cache["nc"]


def kernel(query, key, value, Wq, Wk, Wv, Wo, _trace=False):
    from concourse.bass_utils import run_bass_kernel_spmd

    def t_bf16(x):
        return np.ascontiguousarray(
            np.asarray(x, np.float32).astype(BF16).transpose(0, 2, 1))

    qT = t_bf16(query)
    kT = t_bf16(key)
    vT = t_bf16(value)
    # prescale Wq by the 1/sqrt(HD) attention scale (exact power of 2)
    Wq = (np.asarray(Wq, np.float32) * 0.125).astype(BF16)
    Wk = np.asarray(Wk, np.float32).astype(BF16)
    Wv = np.asarray(Wv, np.float32).astype(BF16)
    Wo = np.asarray(Wo, np.float32).astype(BF16)

    in_maps = []
    for c in range(NCORES):
        in_maps.append({
            "qT": qT, "kT": kT, "vT": vT,
            "wq": np.ascontiguousarray(Wq[:, c * QD:(c + 1) * QD]),
            "wk": np.ascontiguousarray(Wk[:, c * HD:(c + 1) * HD]),
            "wv": np.ascontiguousarray(Wv[:, c * HD:(c + 1) * HD]),
            "wo": np.ascontiguousarray(Wo[c * QD:(c + 1) * QD, :]),
        })

    nc = _get_nc()
    res = run_bass_kernel_spmd(nc, in_maps, list(range(NCORES)), trace=_trace)
    out = res.results[0]["out"].astype(np.float32)
    for c in range(1, NCORES):
        out += res.results[c]["out"].astype(np.float32)
    if _trace:
        _cache["last_exec_time_ns"] = res.exec_time_ns
        _cache["last_results"] = res
    return out


# revision 19
# speedup vs baseline: 1.0745x; 1.0116x over previous
"""GQA kernel for 8 trn2 NeuronCores.

Sharding: tensor-parallel over heads. Core c owns KV head c and Q heads
4c..4c+3 (q-dim cols 256c:256c+256 of Wq, col 64c:64c+64 of Wk/Wv, rows
256c:256c+256 of Wo). Each core computes a partial output [B,S,E]
(its ctx slice @ its Wo row-slice); host sums the 8 partials.

Device algorithm (per core) — v4, software-pipelined across batches:
  A1. Q.T = (Wq/8).T @ X.T as two head-PAIR tiles [128, S] (Wq prescaled
      on host so the PSUM->SBUF eviction is a plain copy).
  A2. K.T [64, S] (dup to 128 partitions) and V natural [S, 64+ones]
      via per-512-col accumulation groups (K rows 0:64, V rows 64:128
      col-packed PSUM) + DMA transposes for V.
  B.  16 units per batch = (q-head h, 512-wide q block j). Per unit,
      a kv-PAIR pipeline (8 steps): two score matmuls into a [128,1024]
      PSUM tile (2 banks), one exp on ScalarE -> pt bf16 [128,1024],
      two ctx matmuls (lagged one pair) accumulating ctx.T[0:65]
      (row 64 = softmax denominator, via ones column in V_aug).
      ScalarE paces the loop; the PE deficit is filled with matmuls from
      A1/A2 of the next batch and phase C of the current batch, pumped
      from filler streams gated by an emission-time DMA-arrival model
      (a filler that would wait on DMA would head-of-line block the
      in-order PE queue).
      Normalize: DVE recip + gpsimd partition-broadcast + DVE mul into
      ctxT tiles [128, 512] per (head-pair, j).
  C.  out_partial[t*128:+128, :] = ctxT.T @ Wo_c, 2-chunk accumulation,
      evicted bf16 and DMA'd out per 128-row tile.

All matmuls bf16 inputs / fp32 PSUM. PSUM banks: sc(2x2) + ctx(2) +
acc(2).
"""

import numpy as np
import ml_dtypes
from collections import deque

B = 2
S = 2048
E = 2048
HD = 64          # head dim
HPC = 4          # q heads per core
QD = HPC * HD    # 256 per-core q dims
NCORES = 8
EC = E // 128    # 16 contraction chunks
NKV = S // 128   # 16 kv chunks of 128
NP = NKV // 2    # 8 kv pairs
NJ = S // 512    # 4 q blocks of 512
BF16 = ml_dtypes.bfloat16

NOTREADY = object()

_cache = {}


def _build():
    from contextlib import ExitStack
    from concourse import bacc, tile
    import concourse.mybir as mybir

    bf16 = mybir.dt.bfloat16
    f32 = mybir.dt.float32
    EXP = mybir.ActivationFunctionType.Exp

    nc = bacc.Bacc(
        "TRN2", target_bir_lowering=False, debug=False, num_devices=NCORES)
    qT_d = nc.declare_dram_parameter("qT", [B, E, S], bf16, isOutput=False)
    kT_d = nc.declare_dram_parameter("kT", [B, E, S], bf16, isOutput=False)
    vT_d = nc.declare_dram_parameter("vT", [B, E, S], bf16, isOutput=False)
    wq_d = nc.declare_dram_parameter("wq", [E, QD], bf16, isOutput=False)
    wk_d = nc.declare_dram_parameter("wk", [E, HD], bf16, isOutput=False)
    wv_d = nc.declare_dram_parameter("wv", [E, HD], bf16, isOutput=False)
    wo_d = nc.declare_dram_parameter("wo", [QD, E], bf16, isOutput=False)
    out_d = nc.declare_dram_parameter("out", [B, S, E], bf16, isOutput=True)

    qT_r = qT_d.rearrange("b (c p) s -> b p c s", p=128)   # [B,128,16,S]
    kT_r = kT_d.rearrange("b (c p) s -> b p c s", p=128)
    vT_r = vT_d.rearrange("b (c p) s -> b p c s", p=128)

    class Sched:
        """Emission-time clock model: pe = estimated wall when the
        instruction being emitted will run; dma = when the DMA queue
        drains. Used only to gate filler emission, not for correctness."""
        NS_PER_BYTE = 1.0 / 360.0e9 * 1e9   # single shared DMA bus

        def __init__(self):
            self.pe = 0.0
            self.dma = 0.0

        def dma_issue(self, nbytes):
            self.dma = max(self.dma, self.pe) + 700 + nbytes * self.NS_PER_BYTE
            return self.dma

    sched = Sched()

    with ExitStack() as ctx:
        tc = ctx.enter_context(tile.TileContext(nc))
        # ---- pools ----
        wpool = ctx.enter_context(tc.tile_pool(name="w", bufs=1))
        qin = ctx.enter_context(tc.tile_pool(name="qin", bufs=2))
        kin = ctx.enter_context(tc.tile_pool(name="kin", bufs=3))
        vin = ctx.enter_context(tc.tile_pool(name="vin", bufs=3))
        qts = ctx.enter_context(tc.tile_pool(name="qts", bufs=4))
        ktp = ctx.enter_context(tc.tile_pool(name="ktp", bufs=2))
        vtp = ctx.enter_context(tc.tile_pool(name="vtp", bufs=2))
        vnp = ctx.enter_context(tc.tile_pool(name="vnp", bufs=1))
        ptp = ctx.enter_context(tc.tile_pool(name="ptp", bufs=3))
        ctp = ctx.enter_context(tc.tile_pool(name="ctp", bufs=1))
        ostp = ctx.enter_context(tc.tile_pool(name="ostp", bufs=2))
        smp = ctx.enter_context(tc.tile_pool(name="smp", bufs=2))
        scp = ctx.enter_context(tc.tile_pool(name="scp", bufs=2, space="PSUM"))
        cxp = ctx.enter_context(tc.tile_pool(name="cxp", bufs=2, space="PSUM"))
        accp = ctx.enter_context(tc.tile_pool(name="accp", bufs=2,
                                              space="PSUM"))

        # ---- weights: wq first, wo deferred (only phase C needs it) ----
        wq_sb = wpool.tile([128, EC, QD], bf16)
        nc.sync.dma_start(wq_sb[:], wq_d.rearrange("(c p) m -> p c m", p=128))
        sched.dma_issue(E * QD * 2)

        def load_q(b):
            quads = []
            for g in range(2):
                qt = qin.tile([128, 8, S], bf16, tag="qt", name=f"q{b}_{g}")
                nc.sync.dma_start(qt[:], qT_r[b, :, 8 * g:8 * g + 8, :])
                quads.append((qt, sched.dma_issue(128 * 8 * S * 2)))
            return quads

        q0 = load_q(0)

        wk_sb = wpool.tile([128, EC, HD], bf16)
        nc.sync.dma_start(wk_sb[:], wk_d.rearrange("(c p) m -> p c m", p=128))
        sched.dma_issue(E * HD * 2)
        wv_sb = wpool.tile([128, EC, HD], bf16)
        nc.sync.dma_start(wv_sb[:], wv_d.rearrange("(c p) m -> p c m", p=128))
        sched.dma_issue(E * HD * 2)
        wo_sb = wpool.tile([128, 2, E], bf16)

        ones64 = wpool.tile([1, 64], bf16)
        nc.vector.memset(ones64[:], 1.0)

        # V_aug tiles, ones column set once (transposes only write 0:HD)
        vn_tiles = [[vnp.tile([128, HD + 1], bf16, name=f"vn{b}_{c}")
                     for c in range(NKV)] for b in range(B)]
        for b in range(B):
            for c in range(NKV):
                nc.vector.memset(vn_tiles[b][c][:, HD:HD + 1], 1.0)

        # per-batch persistent tiles
        qp_sb = {}    # (b, pair) -> Q.T pair tile [128, S]
        kt2_sb = {}   # b -> K.T dup tile [128, S]
        ctxT = {}     # (b, pair, j) -> normalized ctx.T tile [128, 512]

        def a1_stream(b, quads):
            """Q.T projection: 8 groups of 16 accumulating matmuls."""
            qp_sb[(b, 0)] = qts.tile([128, S], bf16, tag="qp", name=f"qp{b}0")
            qp_sb[(b, 1)] = qts.tile([128, S], bf16, tag="qp", name=f"qp{b}1")
            for m in range(2):
                for t in range(NJ):
                    acc = accp.tile([128, 512], f32, tag="acc", name="a1acc")
                    for e in range(EC):
                        qt, ready = quads[e // 8]
                        while sched.pe < ready:
                            yield NOTREADY, ready
                        nc.tensor.matmul(
                            acc[:], lhsT=wq_sb[:, e, m * 128:(m + 1) * 128],
                            rhs=qt[:, e % 8, t * 512:(t + 1) * 512],
                            start=(e == 0), stop=(e == EC - 1))
                        yield None, 0
                    nc.vector.tensor_copy(
                        qp_sb[(b, m)][:, t * 512:(t + 1) * 512], acc[:])

        def load_kv_ti(b, ti):
            quads = []
            for g in range(2):
                kt_in = kin.tile([128, 8, 512], bf16, tag="ki",
                                 name=f"k{b}_{ti}_{g}")
                nc.sync.dma_start(
                    kt_in[:],
                    kT_r[b, :, 8 * g:8 * g + 8, ti * 512:(ti + 1) * 512])
                sched.dma_issue(128 * 8 * 512 * 2)
                vt_in = vin.tile([128, 8, 512], bf16, tag="vi",
                                 name=f"v{b}_{ti}_{g}")
                nc.sync.dma_start(
                    vt_in[:],
                    vT_r[b, :, 8 * g:8 * g + 8, ti * 512:(ti + 1) * 512])
                quads.append((kt_in, vt_in,
                              sched.dma_issue(128 * 8 * 512 * 2)))
            return quads

        def a2_stream(b):
            """K.T / V projections, col block (ti) at a time, with its own
            prefetched input DMAs and V transposes."""
            kt2 = ktp.tile([128, S], bf16, tag="kt2", name=f"kt2_{b}")
            kt2_sb[b] = kt2
            pending = load_kv_ti(b, 0)
            for ti in range(NJ):
                kq = pending
                pending = load_kv_ti(b, ti + 1) if ti + 1 < NJ else None
                acc = accp.tile([128, 512], f32, tag="acc", name="a2acc")
                for e in range(EC):
                    kt_in, vt_in, ready = kq[e // 8]
                    while sched.pe < ready:
                        yield NOTREADY, ready
                    nc.tensor.matmul(
                        acc[0:64, :], lhsT=wk_sb[:, e, :],
                        rhs=kt_in[:, e % 8, :],
                        start=(e == 0), stop=(e == EC - 1))
                    yield None, 0
                    nc.tensor.matmul(
                        acc[64:128, :], lhsT=wv_sb[:, e, :],
                        rhs=vt_in[:, e % 8, :],
                        start=(e == 0), stop=(e == EC - 1),
                        tile_position=(0, 64))
                    yield None, 0
                nc.vector.tensor_copy(
                    kt2[0:64, ti * 512:(ti + 1) * 512], acc[0:64, :])
                vt = vtp.tile([64, 512], bf16, tag="vt", name=f"vt{b}_{ti}")
                nc.vector.tensor_copy(vt[:], acc[64:128, :])
                for c in range(4):
                    nc.sync.dma_start_transpose(
                        out=vn_tiles[b][ti * 4 + c][:, 0:HD],
                        in_=vt[:, c * 128:(c + 1) * 128])
            nc.sync.dma_start(kt2[64:128, :], kt2[0:64, :])

        def c_stream(b, j):
            """Phase C for output rows [j*512, (j+1)*512): 4 row-tiles."""
            for tt in range(4):
                t = j * 4 + tt
                ost = ostp.tile([128, E], bf16, tag="ost", name=f"o{b}_{t}")
                for e in range(4):
                    acc = accp.tile([128, 512], f32, tag="acc", name="cacc")
                    for kc in range(2):
                        nc.tensor.matmul(
                            acc[:],
                            lhsT=ctxT[(b, kc, j)][:, tt * 128:(tt + 1) * 128],
                            rhs=wo_sb[:, kc, e * 512:(e + 1) * 512],
                            start=(kc == 0), stop=(kc == 1))
                        yield None, 0
                    nc.vector.tensor_copy(ost[:, e * 512:(e + 1) * 512],
                                          acc[:])
                nc.sync.dma_start(out_d[b, t * 128:(t + 1) * 128, :], ost[:])
                sched.dma_issue(128 * E * 2)

        # ---- filler machinery ----
        streams = deque()

        def pump(n):
            done = tries = 0
            while done < n and streams and tries < len(streams):
                kind, ready = next(streams[0], ("END", 0))
                if kind is NOTREADY:
                    streams.rotate(-1)
                    tries += 1
                elif kind == "END":
                    streams.popleft()
                else:
                    done += 1
                    tries = 0

        def drain():
            while streams:
                stall = []
                progressed = False
                for _ in range(len(streams)):
                    kind, ready = next(streams[0], ("END", 0))
                    if kind is NOTREADY:
                        stall.append(ready)
                        streams.rotate(-1)
                    elif kind == "END":
                        streams.popleft()
                        progressed = True
                        break
                    else:
                        sched.pe += 213
                        progressed = True
                        break
                if not progressed and stall:
                    sched.pe = max(sched.pe, min(stall))

        def b_unit(b, j, h):
            """Attention for q-head h on q rows [j*512, (j+1)*512)."""
            kc, off = h // 2, (h % 2) * 64
            kt2 = kt2_sb[b]
            qp = qp_sb[(b, kc)]
            ctx_ps = cxp.tile([128, 512], f32, tag="ctx", name=f"cx{b}{j}{h}")
            pend = deque()

            def ctx_pair(p, pt):
                for half in range(2):
                    kv = 2 * p + half
                    nc.tensor.matmul(
                        ctx_ps[0:HD + 1, :],
                        lhsT=vn_tiles[b][kv][:, 0:HD + 1],
                        rhs=pt[:, half * 512:(half + 1) * 512],
                        start=(kv == 0), stop=(kv == NKV - 1))

            for p in range(NP):
                sc = scp.tile([128, 1024], f32, tag="sc", name="sc")
                for half in range(2):
                    kv = 2 * p + half
                    nc.tensor.matmul(
                        sc[:, half * 512:(half + 1) * 512],
                        lhsT=kt2[off:off + 64, kv * 128:(kv + 1) * 128],
                        rhs=qp[off:off + 64, j * 512:(j + 1) * 512],
                        start=True, stop=True)
                pt = ptp.tile([128, 1024], bf16, tag="pt", name="pt")
                nc.scalar.activation(pt[:], sc[:], EXP)
                pend.append((p, pt))
                if len(pend) > 2:
                    ctx_pair(*pend.popleft())
                sched.pe += 1222
                pump(2)
            while pend:
                ctx_pair(*pend.popleft())
            # normalize by softmax denominator (row HD)
            recip = smp.tile([1, 512], f32, tag="recip", name="recip")
            nc.vector.reciprocal(recip[:], ctx_ps[HD:HD + 1, :])
            rb = smp.tile([64, 512], f32, tag="rb", name="rb")
            nc.gpsimd.partition_broadcast(rb[:], recip[:])
            nc.vector.tensor_mul(
                ctxT[(b, kc, j)][off:off + 64, :], ctx_ps[0:64, :], rb[:])
            sched.pe += 2200
            pump(4)

        # ================= schedule =================
        # prolog: batch 0 A phase (DMA-paced)
        streams.append(a1_stream(0, q0))
        streams.append(a2_stream(0))
        drain()
        nc.sync.dma_start(wo_sb[:], wo_d.rearrange("(c p) e -> p c e", p=128))
        sched.dma_issue(QD * E * 2)
        sched.pe = max(sched.pe, sched.dma)

        for b in range(B):
            for kc in range(2):
                for j in range(NJ):
                    ctxT[(b, kc, j)] = ctp.tile(
                        [128, 512], bf16, name=f"ctxT{b}_{kc}_{j}")
            if b + 1 < B:
                qn = load_q(b + 1)
                streams.append(a1_stream(b + 1, qn))
                streams.append(a2_stream(b + 1))
            for j in range(NJ):
                for h in range(HPC):
                    b_unit(b, j, h)
                streams.append(c_stream(b, j))
        drain()

    nc.compile()
    return nc


def _get_nc():
    if "nc" not in _cache:
        _cache["nc"] = _build()
    return _cache["nc"]


def kernel(query, key, value, Wq, Wk, Wv, Wo, _trace=False):
    from concourse.bass_utils import run_bass_kernel_spmd

    def t_bf16(x):
        return np.ascontiguousarray(
            np.asarray(x, np.float32).astype(BF16).transpose(0, 2, 1))

    qT = t_bf16(query)
    kT = t_bf16(key)
    vT = t_bf16(value)
    # prescale Wq by the 1/sqrt(HD) attention scale (exact power of 2)
    Wq = (np.asarray(Wq, np.float32) * 0.125).astype(BF16)
    Wk = np.asarray(Wk, np.float32).astype(BF16)
    Wv = np.asarray(Wv, np.float32).astype(BF16)
    Wo = np.asarray(Wo, np.float32).astype(BF16)

    in_maps = []
    for c in range(NCORES):
        in_maps.append({
            "qT": qT, "kT": kT, "vT": vT,
            "wq": np.ascontiguousarray(Wq[:, c * QD:(c + 1) * QD]),
            "wk": np.ascontiguousarray(Wk[:, c * HD:(c + 1) * HD]),
            "wv": np.ascontiguousarray(Wv[:, c * HD:(c + 1) * HD]),
            "wo": np.ascontiguousarray(Wo[c * QD:(c + 1) * QD, :]),
        })

    nc = _get_nc()
    res = run_bass_kernel_spmd(nc, in_maps, list(range(NCORES)), trace=_trace)
    out = res.results[0]["out"].astype(np.float32)
    for c in range(1, NCORES):
        out += res.results[c]["out"].astype(np.float32)
    if _trace:
        _cache["last_exec_time_ns"] = res.exec_time_ns
        _cache["last_results"] = res
    return out


# revision 24
# speedup vs baseline: 1.0896x; 1.0140x over previous
"""GQA kernel for 8 trn2 NeuronCores.

Sharding: tensor-parallel over heads. Core c owns KV head c and Q heads
4c..4c+3 (q-dim cols 256c:256c+256 of Wq, col 64c:64c+64 of Wk/Wv, rows
256c:256c+256 of Wo). Each core computes a partial output [B,S,E]
(its ctx slice @ its Wo row-slice); host sums the 8 partials.

Device algorithm (per core) — v4, software-pipelined across batches:
  A1. Q.T = (Wq/8).T @ X.T as two head-PAIR tiles [128, S] (Wq prescaled
      on host so the PSUM->SBUF eviction is a plain copy).
  A2. K.T [64, S] (dup to 128 partitions) and V natural [S, 64+ones]
      via per-512-col accumulation groups (K rows 0:64, V rows 64:128
      col-packed PSUM) + DMA transposes for V.
  B.  16 units per batch = (q-head h, 512-wide q block j). Per unit,
      a kv-PAIR pipeline (8 steps): two score matmuls into a [128,1024]
      PSUM tile (2 banks), one exp on ScalarE -> pt bf16 [128,1024],
      two ctx matmuls (lagged one pair) accumulating ctx.T[0:65]
      (row 64 = softmax denominator, via ones column in V_aug).
      ScalarE paces the loop; the PE deficit is filled with matmuls from
      A1/A2 of the next batch and phase C of the current batch, pumped
      from filler streams gated by an emission-time DMA-arrival model
      (a filler that would wait on DMA would head-of-line block the
      in-order PE queue).
      Normalize: DVE recip + gpsimd partition-broadcast + DVE mul into
      ctxT tiles [128, 512] per (head-pair, j).
  C.  out_partial[t*128:+128, :] = ctxT.T @ Wo_c, 2-chunk accumulation,
      evicted bf16 and DMA'd out per 128-row tile.

All matmuls bf16 inputs / fp32 PSUM. PSUM banks: sc(2x2) + ctx(2) +
acc(2).
"""

import numpy as np
import ml_dtypes
from collections import deque

B = 2
S = 2048
E = 2048
HD = 64          # head dim
HPC = 4          # q heads per core
QD = HPC * HD    # 256 per-core q dims
NCORES = 8
EC = E // 128    # 16 contraction chunks
NKV = S // 128   # 16 kv chunks of 128
NP = NKV // 2    # 8 kv pairs
NJ = S // 512    # 4 q blocks of 512
BF16 = ml_dtypes.bfloat16

NOTREADY = object()

_cache = {}


def _build():
    from contextlib import ExitStack
    from concourse import bacc, tile
    import concourse.mybir as mybir

    bf16 = mybir.dt.bfloat16
    f32 = mybir.dt.float32
    EXP = mybir.ActivationFunctionType.Exp

    nc = bacc.Bacc(
        "TRN2", target_bir_lowering=False, debug=False, num_devices=NCORES)
    qT_d = nc.declare_dram_parameter("qT", [B, E, S], bf16, isOutput=False)
    kT_d = nc.declare_dram_parameter("kT", [B, E, S], bf16, isOutput=False)
    vT_d = nc.declare_dram_parameter("vT", [B, E, S], bf16, isOutput=False)
    wq_d = nc.declare_dram_parameter("wq", [E, QD], bf16, isOutput=False)
    wk_d = nc.declare_dram_parameter("wk", [E, HD], bf16, isOutput=False)
    wv_d = nc.declare_dram_parameter("wv", [E, HD], bf16, isOutput=False)
    wo_d = nc.declare_dram_parameter("wo", [QD, E], bf16, isOutput=False)
    out_d = nc.declare_dram_parameter("out", [B, S, E], bf16, isOutput=True)

    qT_r = qT_d.rearrange("b (c p) s -> b p c s", p=128)   # [B,128,16,S]
    kT_r = kT_d.rearrange("b (c p) s -> b p c s", p=128)
    vT_r = vT_d.rearrange("b (c p) s -> b p c s", p=128)

    class Sched:
        """Emission-time clock model: pe = estimated wall when the
        instruction being emitted will run; dma = when the DMA queue
        drains. Used only to gate filler emission, not for correctness."""
        NS_PER_BYTE = 1.0 / 360.0e9 * 1e9   # single shared DMA bus

        def __init__(self):
            self.pe = 0.0
            self.dma = 0.0

        def dma_issue(self, nbytes):
            self.dma = max(self.dma, self.pe) + 700 + nbytes * self.NS_PER_BYTE
            return self.dma

    sched = Sched()

    with ExitStack() as ctx:
        tc = ctx.enter_context(tile.TileContext(nc))
        # ---- pools ----
        wpool = ctx.enter_context(tc.tile_pool(name="w", bufs=1))
        qin = ctx.enter_context(tc.tile_pool(name="qin", bufs=4))
        kin = ctx.enter_context(tc.tile_pool(name="kin", bufs=3))
        vin = ctx.enter_context(tc.tile_pool(name="vin", bufs=3))
        qts = ctx.enter_context(tc.tile_pool(name="qts", bufs=4))
        ktp = ctx.enter_context(tc.tile_pool(name="ktp", bufs=2))
        vtp = ctx.enter_context(tc.tile_pool(name="vtp", bufs=2))
        vnp = ctx.enter_context(tc.tile_pool(name="vnp", bufs=1))
        ptp = ctx.enter_context(tc.tile_pool(name="ptp", bufs=3))
        ctp = ctx.enter_context(tc.tile_pool(name="ctp", bufs=1))
        ostp = ctx.enter_context(tc.tile_pool(name="ostp", bufs=2))
        smp = ctx.enter_context(tc.tile_pool(name="smp", bufs=2))
        scp = ctx.enter_context(tc.tile_pool(name="scp", bufs=2, space="PSUM"))
        cxp = ctx.enter_context(tc.tile_pool(name="cxp", bufs=2, space="PSUM"))
        accp = ctx.enter_context(tc.tile_pool(name="accp", bufs=2,
                                              space="PSUM"))

        # ---- weights: wq first, wo deferred (only phase C needs it) ----
        wq_sb = wpool.tile([128, EC, QD], bf16)
        nc.sync.dma_start(wq_sb[:], wq_d.rearrange("(c p) m -> p c m", p=128))
        sched.dma_issue(E * QD * 2)

        def load_q(b):
            quads = []
            for g in range(4):
                qt = qin.tile([128, 4, S], bf16, tag="qt", name=f"q{b}_{g}")
                nc.sync.dma_start(qt[:], qT_r[b, :, 4 * g:4 * g + 4, :])
                quads.append((qt, sched.dma_issue(128 * 4 * S * 2)))
            return quads

        q0 = load_q(0)

        wk_sb = wpool.tile([128, EC, HD], bf16)
        nc.sync.dma_start(wk_sb[:], wk_d.rearrange("(c p) m -> p c m", p=128))
        sched.dma_issue(E * HD * 2)
        wv_sb = wpool.tile([128, EC, HD], bf16)
        nc.sync.dma_start(wv_sb[:], wv_d.rearrange("(c p) m -> p c m", p=128))
        sched.dma_issue(E * HD * 2)
        wo_sb = wpool.tile([128, 2, E], bf16)

        ones64 = wpool.tile([1, 64], bf16)
        nc.vector.memset(ones64[:], 1.0)

        # V_aug tiles, ones column set once (transposes only write 0:HD)
        vn_tiles = [[vnp.tile([128, HD + 1], bf16, name=f"vn{b}_{c}")
                     for c in range(NKV)] for b in range(B)]
        for b in range(B):
            for c in range(NKV):
                nc.vector.memset(vn_tiles[b][c][:, HD:HD + 1], 1.0)

        # per-batch persistent tiles
        qp_sb = {}    # (b, pair) -> Q.T pair tile [128, S]
        kt2_sb = {}   # b -> K.T dup tile [128, S]
        ctxT = {}     # (b, pair, j) -> normalized ctx.T tile [128, 512]

        def a1_stream(b, quads):
            """Q.T projection: 8 groups of 16 accumulating matmuls."""
            qp_sb[(b, 0)] = qts.tile([128, S], bf16, tag="qp", name=f"qp{b}0")
            qp_sb[(b, 1)] = qts.tile([128, S], bf16, tag="qp", name=f"qp{b}1")
            for m in range(2):
                for t in range(NJ):
                    acc = accp.tile([128, 512], f32, tag="acc", name="a1acc")
                    for e in range(EC):
                        qt, ready = quads[e // 4]
                        while sched.pe < ready:
                            yield NOTREADY, ready
                        nc.tensor.matmul(
                            acc[:], lhsT=wq_sb[:, e, m * 128:(m + 1) * 128],
                            rhs=qt[:, e % 4, t * 512:(t + 1) * 512],
                            start=(e == 0), stop=(e == EC - 1))
                        yield None, 0
                    nc.vector.tensor_copy(
                        qp_sb[(b, m)][:, t * 512:(t + 1) * 512], acc[:])

        def load_kv_ti(b, ti):
            quads = []
            for g in range(2):
                kt_in = kin.tile([128, 8, 512], bf16, tag="ki",
                                 name=f"k{b}_{ti}_{g}")
                nc.sync.dma_start(
                    kt_in[:],
                    kT_r[b, :, 8 * g:8 * g + 8, ti * 512:(ti + 1) * 512])
                sched.dma_issue(128 * 8 * 512 * 2)
                vt_in = vin.tile([128, 8, 512], bf16, tag="vi",
                                 name=f"v{b}_{ti}_{g}")
                nc.sync.dma_start(
                    vt_in[:],
                    vT_r[b, :, 8 * g:8 * g + 8, ti * 512:(ti + 1) * 512])
                quads.append((kt_in, vt_in,
                              sched.dma_issue(128 * 8 * 512 * 2)))
            return quads

        def a2_stream(b):
            """K.T / V projections, col block (ti) at a time, with its own
            prefetched input DMAs and V transposes."""
            kt2 = ktp.tile([128, S], bf16, tag="kt2", name=f"kt2_{b}")
            kt2_sb[b] = kt2
            pending = load_kv_ti(b, 0)
            for ti in range(NJ):
                kq = pending
                pending = load_kv_ti(b, ti + 1) if ti + 1 < NJ else None
                acc = accp.tile([128, 512], f32, tag="acc", name="a2acc")
                for e in range(EC):
                    kt_in, vt_in, ready = kq[e // 8]
                    while sched.pe < ready:
                        yield NOTREADY, ready
                    nc.tensor.matmul(
                        acc[0:64, :], lhsT=wk_sb[:, e, :],
                        rhs=kt_in[:, e % 8, :],
                        start=(e == 0), stop=(e == EC - 1))
                    yield None, 0
                    nc.tensor.matmul(
                        acc[64:128, :], lhsT=wv_sb[:, e, :],
                        rhs=vt_in[:, e % 8, :],
                        start=(e == 0), stop=(e == EC - 1),
                        tile_position=(0, 64))
                    yield None, 0
                nc.vector.tensor_copy(
                    kt2[0:64, ti * 512:(ti + 1) * 512], acc[0:64, :])
                vt = vtp.tile([64, 512], bf16, tag="vt", name=f"vt{b}_{ti}")
                nc.vector.tensor_copy(vt[:], acc[64:128, :])
                for c in range(4):
                    nc.sync.dma_start_transpose(
                        out=vn_tiles[b][ti * 4 + c][:, 0:HD],
                        in_=vt[:, c * 128:(c + 1) * 128])
            nc.sync.dma_start(kt2[64:128, :], kt2[0:64, :])

        def c_stream(b, j):
            """Phase C for output rows [j*512, (j+1)*512): 4 row-tiles."""
            for tt in range(4):
                t = j * 4 + tt
                ost = ostp.tile([128, E], bf16, tag="ost", name=f"o{b}_{t}")
                for e in range(4):
                    acc = accp.tile([128, 512], f32, tag="acc", name="cacc")
                    for kc in range(2):
                        nc.tensor.matmul(
                            acc[:],
                            lhsT=ctxT[(b, kc, j)][:, tt * 128:(tt + 1) * 128],
                            rhs=wo_sb[:, kc, e * 512:(e + 1) * 512],
                            start=(kc == 0), stop=(kc == 1))
                        yield None, 0
                    nc.vector.tensor_copy(ost[:, e * 512:(e + 1) * 512],
                                          acc[:])
                nc.sync.dma_start(out_d[b, t * 128:(t + 1) * 128, :], ost[:])
                sched.dma_issue(128 * E * 2)

        # ---- filler machinery ----
        streams = deque()

        def pump(n):
            done = tries = 0
            while done < n and streams and tries < len(streams):
                kind, ready = next(streams[0], ("END", 0))
                if kind is NOTREADY:
                    streams.rotate(-1)
                    tries += 1
                elif kind == "END":
                    streams.popleft()
                else:
                    done += 1
                    tries = 0

        def drain():
            while streams:
                stall = []
                progressed = False
                for _ in range(len(streams)):
                    kind, ready = next(streams[0], ("END", 0))
                    if kind is NOTREADY:
                        stall.append(ready)
                        streams.rotate(-1)
                    elif kind == "END":
                        streams.popleft()
                        progressed = True
                        break
                    else:
                        sched.pe += 213
                        progressed = True
                        break
                if not progressed and stall:
                    sched.pe = max(sched.pe, min(stall))

        def b_unit(b, j, h):
            """Attention for q-head h on q rows [j*512, (j+1)*512)."""
            kc, off = h // 2, (h % 2) * 64
            kt2 = kt2_sb[b]
            qp = qp_sb[(b, kc)]
            ctx_ps = cxp.tile([128, 512], f32, tag="ctx", name=f"cx{b}{j}{h}")
            pend = deque()

            def ctx_pair(p, pt):
                for half in range(2):
                    kv = 2 * p + half
                    nc.tensor.matmul(
                        ctx_ps[0:HD + 1, :],
                        lhsT=vn_tiles[b][kv][:, 0:HD + 1],
                        rhs=pt[:, half * 512:(half + 1) * 512],
                        start=(kv == 0), stop=(kv == NKV - 1))

            for p in range(NP):
                sc = scp.tile([128, 1024], f32, tag="sc", name="sc")
                for half in range(2):
                    kv = 2 * p + half
                    nc.tensor.matmul(
                        sc[:, half * 512:(half + 1) * 512],
                        lhsT=kt2[off:off + 64, kv * 128:(kv + 1) * 128],
                        rhs=qp[off:off + 64, j * 512:(j + 1) * 512],
                        start=True, stop=True)
                pt = ptp.tile([128, 1024], bf16, tag="pt", name="pt")
                nc.scalar.activation(pt[:], sc[:], EXP)
                pend.append((p, pt))
                if len(pend) > 2:
                    ctx_pair(*pend.popleft())
                sched.pe += 1222
                pump(2)
            while pend:
                ctx_pair(*pend.popleft())
            # normalize by softmax denominator (row HD)
            recip = smp.tile([1, 512], f32, tag="recip", name="recip")
            nc.vector.reciprocal(recip[:], ctx_ps[HD:HD + 1, :])
            rb = smp.tile([64, 512], f32, tag="rb", name="rb")
            nc.gpsimd.partition_broadcast(rb[:], recip[:])
            nc.vector.tensor_mul(
                ctxT[(b, kc, j)][off:off + 64, :], ctx_ps[0:64, :], rb[:])
            sched.pe += 2200
            pump(4)

        # ================= schedule =================
        # prolog: batch 0 A phase (DMA-paced)
        streams.append(a1_stream(0, q0))
        streams.append(a2_stream(0))
        drain()
        nc.sync.dma_start(wo_sb[:], wo_d.rearrange("(c p) e -> p c e", p=128))
        sched.dma_issue(QD * E * 2)
        sched.pe = max(sched.pe, sched.dma)

        carry_c = []
        for b in range(B):
            for kc in range(2):
                for j in range(NJ):
                    ctxT[(b, kc, j)] = ctp.tile(
                        [128, 512], bf16, name=f"ctxT{b}_{kc}_{j}")
            if b + 1 < B:
                qn = load_q(b + 1)
                streams.append(a1_stream(b + 1, qn))
                streams.append(a2_stream(b + 1))
            for g in carry_c:
                streams.append(g)
            carry_c = []
            for j in range(NJ):
                for h in range(HPC):
                    b_unit(b, j, h)
                # later j-segments of this batch's phase C fill the NEXT
                # batch's B window (this batch's window is already fed by
                # A of b+1); the last batch keeps its own C.
                if b + 1 < B and j >= 2:
                    carry_c.append(c_stream(b, j))
                else:
                    streams.append(c_stream(b, j))
        drain()

    nc.compile()
    return nc


def _get_nc():
    if "nc" not in _cache:
        _cache["nc"] = _build()
    return _cache["nc"]


def kernel(query, key, value, Wq, Wk, Wv, Wo, _trace=False):
    from concourse.bass_utils import run_bass_kernel_spmd

    def t_bf16(x):
        return np.ascontiguousarray(
            np.asarray(x, np.float32).astype(BF16).transpose(0, 2, 1))

    qT = t_bf16(query)
    kT = t_bf16(key)
    vT = t_bf16(value)
    # prescale Wq by the 1/sqrt(HD) attention scale (exact power of 2)
    Wq = (np.asarray(Wq, np.float32) * 0.125).astype(BF16)
    Wk = np.asarray(Wk, np.float32).astype(BF16)
    Wv = np.asarray(Wv, np.float32).astype(BF16)
    Wo = np.asarray(Wo, np.float32).astype(BF16)

    in_maps = []
    for c in range(NCORES):
        in_maps.append({
            "qT": qT, "kT": kT, "vT": vT,
            "wq": np.ascontiguousarray(Wq[:, c * QD:(c + 1) * QD]),
            "wk": np.ascontiguousarray(Wk[:, c * HD:(c + 1) * HD]),
            "wv": np.ascontiguousarray(Wv[:, c * HD:(c + 1) * HD]),
            "wo": np.ascontiguousarray(Wo[c * QD:(c + 1) * QD, :]),
        })

    nc = _get_nc()
    res = run_bass_kernel_spmd(nc, in_maps, list(range(NCORES)), trace=_trace)
    out = res.results[0]["out"].astype(np.float32)
    for c in range(1, NCORES):
        out += res.results[c]["out"].astype(np.float32)
    if _trace:
        _cache["last_exec_time_ns"] = res.exec_time_ns
        _cache["last_results"] = res
    return out


# revision 25
# speedup vs baseline: 1.0921x; 1.0023x over previous
"""GQA kernel for 8 trn2 NeuronCores.

Sharding: tensor-parallel over heads. Core c owns KV head c and Q heads
4c..4c+3 (q-dim cols 256c:256c+256 of Wq, col 64c:64c+64 of Wk/Wv, rows
256c:256c+256 of Wo). Each core computes a partial output [B,S,E]
(its ctx slice @ its Wo row-slice); host sums the 8 partials.

Device algorithm (per core) — v4, software-pipelined across batches:
  A1. Q.T = (Wq/8).T @ X.T as two head-PAIR tiles [128, S] (Wq prescaled
      on host so the PSUM->SBUF eviction is a plain copy).
  A2. K.T [64, S] (dup to 128 partitions) and V natural [S, 64+ones]
      via per-512-col accumulation groups (K rows 0:64, V rows 64:128
      col-packed PSUM) + DMA transposes for V.
  B.  16 units per batch = (q-head h, 512-wide q block j). Per unit,
      a kv-PAIR pipeline (8 steps): two score matmuls into a [128,1024]
      PSUM tile (2 banks), one exp on ScalarE -> pt bf16 [128,1024],
      two ctx matmuls (lagged one pair) accumulating ctx.T[0:65]
      (row 64 = softmax denominator, via ones column in V_aug).
      ScalarE paces the loop; the PE deficit is filled with matmuls from
      A1/A2 of the next batch and phase C of the current batch, pumped
      from filler streams gated by an emission-time DMA-arrival model
      (a filler that would wait on DMA would head-of-line block the
      in-order PE queue).
      Normalize: DVE recip + gpsimd partition-broadcast + DVE mul into
      ctxT tiles [128, 512] per (head-pair, j).
  C.  out_partial[t*128:+128, :] = ctxT.T @ Wo_c, 2-chunk accumulation,
      evicted bf16 and DMA'd out per 128-row tile.

All matmuls bf16 inputs / fp32 PSUM. PSUM banks: sc(2x2) + ctx(2) +
acc(2).
"""

import numpy as np
import ml_dtypes
from collections import deque

B = 2
S = 2048
E = 2048
HD = 64          # head dim
HPC = 4          # q heads per core
QD = HPC * HD    # 256 per-core q dims
NCORES = 8
EC = E // 128    # 16 contraction chunks
NKV = S // 128   # 16 kv chunks of 128
NP = NKV // 2    # 8 kv pairs
NJ = S // 512    # 4 q blocks of 512
BF16 = ml_dtypes.bfloat16

NOTREADY = object()

_cache = {}


def _build():
    from contextlib import ExitStack
    from concourse import bacc, tile
    import concourse.mybir as mybir

    bf16 = mybir.dt.bfloat16
    f32 = mybir.dt.float32
    EXP = mybir.ActivationFunctionType.Exp

    nc = bacc.Bacc(
        "TRN2", target_bir_lowering=False, debug=False, num_devices=NCORES)
    qT_d = nc.declare_dram_parameter("qT", [B, E, S], bf16, isOutput=False)
    kT_d = nc.declare_dram_parameter("kT", [B, E, S], bf16, isOutput=False)
    vT_d = nc.declare_dram_parameter("vT", [B, E, S], bf16, isOutput=False)
    wq_d = nc.declare_dram_parameter("wq", [E, QD], bf16, isOutput=False)
    wk_d = nc.declare_dram_parameter("wk", [E, HD], bf16, isOutput=False)
    wv_d = nc.declare_dram_parameter("wv", [E, HD], bf16, isOutput=False)
    wo_d = nc.declare_dram_parameter("wo", [QD, E], bf16, isOutput=False)
    out_d = nc.declare_dram_parameter("out", [B, S, E], bf16, isOutput=True)

    qT_r = qT_d.rearrange("b (c p) s -> b p c s", p=128)   # [B,128,16,S]
    kT_r = kT_d.rearrange("b (c p) s -> b p c s", p=128)
    vT_r = vT_d.rearrange("b (c p) s -> b p c s", p=128)

    class Sched:
        """Emission-time clock model: pe = estimated wall when the
        instruction being emitted will run; dma = when the DMA queue
        drains. Used only to gate filler emission, not for correctness."""
        NS_PER_BYTE = 1.0 / 360.0e9 * 1e9   # single shared DMA bus

        def __init__(self):
            self.pe = 0.0
            self.dma = 0.0

        def dma_issue(self, nbytes):
            self.dma = max(self.dma, self.pe) + 700 + nbytes * self.NS_PER_BYTE
            return self.dma

    sched = Sched()

    with ExitStack() as ctx:
        tc = ctx.enter_context(tile.TileContext(nc))
        # ---- pools ----
        wpool = ctx.enter_context(tc.tile_pool(name="w", bufs=1))
        qin = ctx.enter_context(tc.tile_pool(name="qin", bufs=4))
        kin = ctx.enter_context(tc.tile_pool(name="kin", bufs=3))
        vin = ctx.enter_context(tc.tile_pool(name="vin", bufs=3))
        qts = ctx.enter_context(tc.tile_pool(name="qts", bufs=4))
        ktp = ctx.enter_context(tc.tile_pool(name="ktp", bufs=2))
        vtp = ctx.enter_context(tc.tile_pool(name="vtp", bufs=2))
        vnp = ctx.enter_context(tc.tile_pool(name="vnp", bufs=1))
        ptp = ctx.enter_context(tc.tile_pool(name="ptp", bufs=3))
        ctp = ctx.enter_context(tc.tile_pool(name="ctp", bufs=1))
        ostp = ctx.enter_context(tc.tile_pool(name="ostp", bufs=2))
        smp = ctx.enter_context(tc.tile_pool(name="smp", bufs=2))
        scp = ctx.enter_context(tc.tile_pool(name="scp", bufs=2, space="PSUM"))
        cxp = ctx.enter_context(tc.tile_pool(name="cxp", bufs=2, space="PSUM"))
        accp = ctx.enter_context(tc.tile_pool(name="accp", bufs=2,
                                              space="PSUM"))

        # ---- weights: wq first, wo deferred (only phase C needs it) ----
        wq_sb = wpool.tile([128, EC, QD], bf16)
        nc.sync.dma_start(wq_sb[:], wq_d.rearrange("(c p) m -> p c m", p=128))
        sched.dma_issue(E * QD * 2)

        def load_q(b):
            quads = []
            for g in range(4):
                qt = qin.tile([128, 4, S], bf16, tag="qt", name=f"q{b}_{g}")
                nc.sync.dma_start(qt[:], qT_r[b, :, 4 * g:4 * g + 4, :])
                quads.append((qt, sched.dma_issue(128 * 4 * S * 2)))
            return quads

        q0 = load_q(0)

        wk_sb = wpool.tile([128, EC, HD], bf16)
        nc.sync.dma_start(wk_sb[:], wk_d.rearrange("(c p) m -> p c m", p=128))
        sched.dma_issue(E * HD * 2)
        wv_sb = wpool.tile([128, EC, HD], bf16)
        nc.sync.dma_start(wv_sb[:], wv_d.rearrange("(c p) m -> p c m", p=128))
        sched.dma_issue(E * HD * 2)
        wo_sb = wpool.tile([128, 2, E], bf16)

        ones64 = wpool.tile([1, 64], bf16)
        nc.vector.memset(ones64[:], 1.0)

        # V_aug tiles, ones column set once (transposes only write 0:HD)
        vn_tiles = [[vnp.tile([128, HD + 1], bf16, name=f"vn{b}_{c}")
                     for c in range(NKV)] for b in range(B)]
        for b in range(B):
            for c in range(NKV):
                nc.vector.memset(vn_tiles[b][c][:, HD:HD + 1], 1.0)

        # per-batch persistent tiles
        qp_sb = {}    # (b, pair) -> Q.T pair tile [128, S]
        kt2_sb = {}   # b -> K.T dup tile [128, S]
        ctxT = {}     # (b, pair, j) -> normalized ctx.T tile [128, 512]

        def a1_stream(b, quads):
            """Q.T projection: 8 groups of 16 accumulating matmuls."""
            qp_sb[(b, 0)] = qts.tile([128, S], bf16, tag="qp", name=f"qp{b}0")
            qp_sb[(b, 1)] = qts.tile([128, S], bf16, tag="qp", name=f"qp{b}1")
            for m in range(2):
                for t in range(NJ):
                    acc = accp.tile([128, 512], f32, tag="acc", name="a1acc")
                    for e in range(EC):
                        qt, ready = quads[e // 4]
                        while sched.pe < ready:
                            yield NOTREADY, ready
                        nc.tensor.matmul(
                            acc[:], lhsT=wq_sb[:, e, m * 128:(m + 1) * 128],
                            rhs=qt[:, e % 4, t * 512:(t + 1) * 512],
                            start=(e == 0), stop=(e == EC - 1))
                        yield None, 0
                    nc.vector.tensor_copy(
                        qp_sb[(b, m)][:, t * 512:(t + 1) * 512], acc[:])

        def load_kv_ti(b, ti):
            quads = []
            for g in range(2):
                kt_in = kin.tile([128, 8, 512], bf16, tag="ki",
                                 name=f"k{b}_{ti}_{g}")
                nc.sync.dma_start(
                    kt_in[:],
                    kT_r[b, :, 8 * g:8 * g + 8, ti * 512:(ti + 1) * 512])
                sched.dma_issue(128 * 8 * 512 * 2)
                vt_in = vin.tile([128, 8, 512], bf16, tag="vi",
                                 name=f"v{b}_{ti}_{g}")
                nc.sync.dma_start(
                    vt_in[:],
                    vT_r[b, :, 8 * g:8 * g + 8, ti * 512:(ti + 1) * 512])
                quads.append((kt_in, vt_in,
                              sched.dma_issue(128 * 8 * 512 * 2)))
            return quads

        def a2_stream(b):
            """K.T / V projections, col block (ti) at a time, with its own
            prefetched input DMAs and V transposes."""
            kt2 = ktp.tile([128, S], bf16, tag="kt2", name=f"kt2_{b}")
            kt2_sb[b] = kt2
            pending = load_kv_ti(b, 0)
            for ti in range(NJ):
                kq = pending
                pending = load_kv_ti(b, ti + 1) if ti + 1 < NJ else None
                acc = accp.tile([128, 512], f32, tag="acc", name="a2acc")
                for e in range(EC):
                    kt_in, vt_in, ready = kq[e // 8]
                    while sched.pe < ready:
                        yield NOTREADY, ready
                    nc.tensor.matmul(
                        acc[0:64, :], lhsT=wk_sb[:, e, :],
                        rhs=kt_in[:, e % 8, :],
                        start=(e == 0), stop=(e == EC - 1))
                    yield None, 0
                    nc.tensor.matmul(
                        acc[64:128, :], lhsT=wv_sb[:, e, :],
                        rhs=vt_in[:, e % 8, :],
                        start=(e == 0), stop=(e == EC - 1),
                        tile_position=(0, 64))
                    yield None, 0
                nc.vector.tensor_copy(
                    kt2[0:64, ti * 512:(ti + 1) * 512], acc[0:64, :])
                # duplicate K.T into partitions 64:128 per-slice so the
                # last write to kt2 lands right after the last eviction
                nc.sync.dma_start(kt2[64:128, ti * 512:(ti + 1) * 512],
                                  kt2[0:64, ti * 512:(ti + 1) * 512])
                vt = vtp.tile([64, 512], bf16, tag="vt", name=f"vt{b}_{ti}")
                nc.vector.tensor_copy(vt[:], acc[64:128, :])
                for c in range(4):
                    nc.sync.dma_start_transpose(
                        out=vn_tiles[b][ti * 4 + c][:, 0:HD],
                        in_=vt[:, c * 128:(c + 1) * 128])

        def c_stream(b, j):
            """Phase C for output rows [j*512, (j+1)*512): 4 row-tiles."""
            for tt in range(4):
                t = j * 4 + tt
                ost = ostp.tile([128, E], bf16, tag="ost", name=f"o{b}_{t}")
                for e in range(4):
                    acc = accp.tile([128, 512], f32, tag="acc", name="cacc")
                    for kc in range(2):
                        nc.tensor.matmul(
                            acc[:],
                            lhsT=ctxT[(b, kc, j)][:, tt * 128:(tt + 1) * 128],
                            rhs=wo_sb[:, kc, e * 512:(e + 1) * 512],
                            start=(kc == 0), stop=(kc == 1))
                        yield None, 0
                    nc.vector.tensor_copy(ost[:, e * 512:(e + 1) * 512],
                                          acc[:])
                nc.sync.dma_start(out_d[b, t * 128:(t + 1) * 128, :], ost[:])
                sched.dma_issue(128 * E * 2)

        # ---- filler machinery ----
        streams = deque()

        def pump(n):
            done = tries = 0
            while done < n and streams and tries < len(streams):
                kind, ready = next(streams[0], ("END", 0))
                if kind is NOTREADY:
                    streams.rotate(-1)
                    tries += 1
                elif kind == "END":
                    streams.popleft()
                else:
                    done += 1
                    tries = 0

        def drain():
            while streams:
                stall = []
                progressed = False
                for _ in range(len(streams)):
                    kind, ready = next(streams[0], ("END", 0))
                    if kind is NOTREADY:
                        stall.append(ready)
                        streams.rotate(-1)
                    elif kind == "END":
                        streams.popleft()
                        progressed = True
                        break
                    else:
                        sched.pe += 213
                        progressed = True
                        break
                if not progressed and stall:
                    sched.pe = max(sched.pe, min(stall))

        def b_unit(b, j, h):
            """Attention for q-head h on q rows [j*512, (j+1)*512)."""
            kc, off = h // 2, (h % 2) * 64
            kt2 = kt2_sb[b]
            qp = qp_sb[(b, kc)]
            ctx_ps = cxp.tile([128, 512], f32, tag="ctx", name=f"cx{b}{j}{h}")
            pend = deque()

            def ctx_pair(p, pt):
                for half in range(2):
                    kv = 2 * p + half
                    nc.tensor.matmul(
                        ctx_ps[0:HD + 1, :],
                        lhsT=vn_tiles[b][kv][:, 0:HD + 1],
                        rhs=pt[:, half * 512:(half + 1) * 512],
                        start=(kv == 0), stop=(kv == NKV - 1))

            for p in range(NP):
                sc = scp.tile([128, 1024], f32, tag="sc", name="sc")
                for half in range(2):
                    kv = 2 * p + half
                    nc.tensor.matmul(
                        sc[:, half * 512:(half + 1) * 512],
                        lhsT=kt2[off:off + 64, kv * 128:(kv + 1) * 128],
                        rhs=qp[off:off + 64, j * 512:(j + 1) * 512],
                        start=True, stop=True)
                pt = ptp.tile([128, 1024], bf16, tag="pt", name="pt")
                nc.scalar.activation(pt[:], sc[:], EXP)
                pend.append((p, pt))
                if len(pend) > 2:
                    ctx_pair(*pend.popleft())
                sched.pe += 1222
                pump(2)
            while pend:
                ctx_pair(*pend.popleft())
            # normalize by softmax denominator (row HD)
            recip = smp.tile([1, 512], f32, tag="recip", name="recip")
            nc.vector.reciprocal(recip[:], ctx_ps[HD:HD + 1, :])
            rb = smp.tile([64, 512], f32, tag="rb", name="rb")
            nc.gpsimd.partition_broadcast(rb[:], recip[:])
            nc.vector.tensor_mul(
                ctxT[(b, kc, j)][off:off + 64, :], ctx_ps[0:64, :], rb[:])
            sched.pe += 2200
            pump(4)

        # ================= schedule =================
        # prolog: batch 0 A phase (DMA-paced)
        streams.append(a1_stream(0, q0))
        streams.append(a2_stream(0))
        drain()
        nc.sync.dma_start(wo_sb[:], wo_d.rearrange("(c p) e -> p c e", p=128))
        sched.dma_issue(QD * E * 2)
        sched.pe = max(sched.pe, sched.dma)

        carry_c = []
        for b in range(B):
            for kc in range(2):
                for j in range(NJ):
                    ctxT[(b, kc, j)] = ctp.tile(
                        [128, 512], bf16, name=f"ctxT{b}_{kc}_{j}")
            if b + 1 < B:
                qn = load_q(b + 1)
                streams.append(a1_stream(b + 1, qn))
                streams.append(a2_stream(b + 1))
            for g in carry_c:
                streams.append(g)
            carry_c = []
            for j in range(NJ):
                for h in range(HPC):
                    b_unit(b, j, h)
                # later j-segments of this batch's phase C fill the NEXT
                # batch's B window (this batch's window is already fed by
                # A of b+1); the last batch keeps its own C.
                if b + 1 < B and j >= 2:
                    carry_c.append(c_stream(b, j))
                else:
                    streams.append(c_stream(b, j))
        drain()

    nc.compile()
    return nc


def _get_nc():
    if "nc" not in _cache:
        _cache["nc"] = _build()
    return _cache["nc"]


def kernel(query, key, value, Wq, Wk, Wv, Wo, _trace=False):
    from concourse.bass_utils import run_bass_kernel_spmd

    def t_bf16(x):
        return np.ascontiguousarray(
            np.asarray(x, np.float32).astype(BF16).transpose(0, 2, 1))

    qT = t_bf16(query)
    kT = t_bf16(key)
    vT = t_bf16(value)
    # prescale Wq by the 1/sqrt(HD) attention scale (exact power of 2)
    Wq = (np.asarray(Wq, np.float32) * 0.125).astype(BF16)
    Wk = np.asarray(Wk, np.float32).astype(BF16)
    Wv = np.asarray(Wv, np.float32).astype(BF16)
    Wo = np.asarray(Wo, np.float32).astype(BF16)

    in_maps = []
    for c in range(NCORES):
        in_maps.append({
            "qT": qT, "kT": kT, "vT": vT,
            "wq": np.ascontiguousarray(Wq[:, c * QD:(c + 1) * QD]),
            "wk": np.ascontiguousarray(Wk[:, c * HD:(c + 1) * HD]),
            "wv": np.ascontiguousarray(Wv[:, c * HD:(c + 1) * HD]),
            "wo": np.ascontiguousarray(Wo[c * QD:(c + 1) * QD, :]),
        })

    nc = _get_nc()
    res = run_bass_kernel_spmd(nc, in_maps, list(range(NCORES)), trace=_trace)
    out = res.results[0]["out"].astype(np.float32)
    for c in range(1, NCORES):
        out += res.results[c]["out"].astype(np.float32)
    if _trace:
        _cache["last_exec_time_ns"] = res.exec_time_ns
        _cache["last_results"] = res
    return out


# revision 26
# speedup vs baseline: 1.0939x; 1.0016x over previous
"""GQA kernel for 8 trn2 NeuronCores.

Sharding: tensor-parallel over heads. Core c owns KV head c and Q heads
4c..4c+3 (q-dim cols 256c:256c+256 of Wq, col 64c:64c+64 of Wk/Wv, rows
256c:256c+256 of Wo). Each core computes a partial output [B,S,E]
(its ctx slice @ its Wo row-slice); host sums the 8 partials.

Device algorithm (per core) — v4, software-pipelined across batches:
  A1. Q.T = (Wq/8).T @ X.T as two head-PAIR tiles [128, S] (Wq prescaled
      on host so the PSUM->SBUF eviction is a plain copy).
  A2. K.T [64, S] (dup to 128 partitions) and V natural [S, 64+ones]
      via per-512-col accumulation groups (K rows 0:64, V rows 64:128
      col-packed PSUM) + DMA transposes for V.
  B.  16 units per batch = (q-head h, 512-wide q block j). Per unit,
      a kv-PAIR pipeline (8 steps): two score matmuls into a [128,1024]
      PSUM tile (2 banks), one exp on ScalarE -> pt bf16 [128,1024],
      two ctx matmuls (lagged one pair) accumulating ctx.T[0:65]
      (row 64 = softmax denominator, via ones column in V_aug).
      ScalarE paces the loop; the PE deficit is filled with matmuls from
      A1/A2 of the next batch and phase C of the current batch, pumped
      from filler streams gated by an emission-time DMA-arrival model
      (a filler that would wait on DMA would head-of-line block the
      in-order PE queue).
      Normalize: DVE recip + gpsimd partition-broadcast + DVE mul into
      ctxT tiles [128, 512] per (head-pair, j).
  C.  out_partial[t*128:+128, :] = ctxT.T @ Wo_c, 2-chunk accumulation,
      evicted bf16 and DMA'd out per 128-row tile.

All matmuls bf16 inputs / fp32 PSUM. PSUM banks: sc(2x2) + ctx(2) +
acc(2).
"""

import numpy as np
import ml_dtypes
from collections import deque

B = 2
S = 2048
E = 2048
HD = 64          # head dim
HPC = 4          # q heads per core
QD = HPC * HD    # 256 per-core q dims
NCORES = 8
EC = E // 128    # 16 contraction chunks
NKV = S // 128   # 16 kv chunks of 128
NP = NKV // 2    # 8 kv pairs
NJ = S // 512    # 4 q blocks of 512
BF16 = ml_dtypes.bfloat16

NOTREADY = object()

_cache = {}


def _build():
    from contextlib import ExitStack
    from concourse import bacc, tile
    import concourse.mybir as mybir

    bf16 = mybir.dt.bfloat16
    f32 = mybir.dt.float32
    EXP = mybir.ActivationFunctionType.Exp

    nc = bacc.Bacc(
        "TRN2", target_bir_lowering=False, debug=False, num_devices=NCORES)
    qT_d = nc.declare_dram_parameter("qT", [B, E, S], bf16, isOutput=False)
    kT_d = nc.declare_dram_parameter("kT", [B, E, S], bf16, isOutput=False)
    vT_d = nc.declare_dram_parameter("vT", [B, E, S], bf16, isOutput=False)
    wq_d = nc.declare_dram_parameter("wq", [E, QD], bf16, isOutput=False)
    wk_d = nc.declare_dram_parameter("wk", [E, HD], bf16, isOutput=False)
    wv_d = nc.declare_dram_parameter("wv", [E, HD], bf16, isOutput=False)
    wo_d = nc.declare_dram_parameter("wo", [QD, E], bf16, isOutput=False)
    out_d = nc.declare_dram_parameter("out", [B, S, E], bf16, isOutput=True)

    qT_r = qT_d.rearrange("b (c p) s -> b p c s", p=128)   # [B,128,16,S]
    kT_r = kT_d.rearrange("b (c p) s -> b p c s", p=128)
    vT_r = vT_d.rearrange("b (c p) s -> b p c s", p=128)

    class Sched:
        """Emission-time clock model: pe = estimated wall when the
        instruction being emitted will run; dma = when the DMA queue
        drains. Used only to gate filler emission, not for correctness."""
        NS_PER_BYTE = 1.0 / 360.0e9 * 1e9   # single shared DMA bus

        def __init__(self):
            self.pe = 0.0
            self.dma = 0.0

        MARGIN = 2500.0   # gate slack: bus queueing the model can't see

        def dma_issue(self, nbytes):
            self.dma = (max(self.dma, self.pe) + 1050
                        + nbytes * self.NS_PER_BYTE)
            return self.dma + self.MARGIN

    sched = Sched()

    with ExitStack() as ctx:
        tc = ctx.enter_context(tile.TileContext(nc))
        # ---- pools ----
        wpool = ctx.enter_context(tc.tile_pool(name="w", bufs=1))
        qin = ctx.enter_context(tc.tile_pool(name="qin", bufs=4))
        kin = ctx.enter_context(tc.tile_pool(name="kin", bufs=3))
        vin = ctx.enter_context(tc.tile_pool(name="vin", bufs=3))
        qts = ctx.enter_context(tc.tile_pool(name="qts", bufs=4))
        ktp = ctx.enter_context(tc.tile_pool(name="ktp", bufs=2))
        vtp = ctx.enter_context(tc.tile_pool(name="vtp", bufs=2))
        vnp = ctx.enter_context(tc.tile_pool(name="vnp", bufs=1))
        ptp = ctx.enter_context(tc.tile_pool(name="ptp", bufs=3))
        ctp = ctx.enter_context(tc.tile_pool(name="ctp", bufs=1))
        ostp = ctx.enter_context(tc.tile_pool(name="ostp", bufs=2))
        smp = ctx.enter_context(tc.tile_pool(name="smp", bufs=2))
        scp = ctx.enter_context(tc.tile_pool(name="scp", bufs=2, space="PSUM"))
        cxp = ctx.enter_context(tc.tile_pool(name="cxp", bufs=2, space="PSUM"))
        accp = ctx.enter_context(tc.tile_pool(name="accp", bufs=2,
                                              space="PSUM"))

        # ---- weights: wq first, wo deferred (only phase C needs it) ----
        wq_sb = wpool.tile([128, EC, QD], bf16)
        nc.sync.dma_start(wq_sb[:], wq_d.rearrange("(c p) m -> p c m", p=128))
        sched.dma_issue(E * QD * 2)

        def load_q(b):
            quads = []
            for g in range(4):
                qt = qin.tile([128, 4, S], bf16, tag="qt", name=f"q{b}_{g}")
                nc.sync.dma_start(qt[:], qT_r[b, :, 4 * g:4 * g + 4, :])
                quads.append((qt, sched.dma_issue(128 * 4 * S * 2)))
            return quads

        q0 = load_q(0)

        wk_sb = wpool.tile([128, EC, HD], bf16)
        nc.sync.dma_start(wk_sb[:], wk_d.rearrange("(c p) m -> p c m", p=128))
        sched.dma_issue(E * HD * 2)
        wv_sb = wpool.tile([128, EC, HD], bf16)
        nc.sync.dma_start(wv_sb[:], wv_d.rearrange("(c p) m -> p c m", p=128))
        sched.dma_issue(E * HD * 2)
        wo_sb = wpool.tile([128, 2, E], bf16)

        ones64 = wpool.tile([1, 64], bf16)
        nc.vector.memset(ones64[:], 1.0)

        # V_aug tiles, ones column set once (transposes only write 0:HD)
        vn_tiles = [[vnp.tile([128, HD + 1], bf16, name=f"vn{b}_{c}")
                     for c in range(NKV)] for b in range(B)]
        for b in range(B):
            for c in range(NKV):
                nc.vector.memset(vn_tiles[b][c][:, HD:HD + 1], 1.0)

        # per-batch persistent tiles
        qp_sb = {}    # (b, pair) -> Q.T pair tile [128, S]
        kt2_sb = {}   # b -> K.T dup tile [128, S]
        ctxT = {}     # (b, pair, j) -> normalized ctx.T tile [128, 512]

        def a1_stream(b, quads):
            """Q.T projection: 8 groups of 16 accumulating matmuls."""
            qp_sb[(b, 0)] = qts.tile([128, S], bf16, tag="qp", name=f"qp{b}0")
            qp_sb[(b, 1)] = qts.tile([128, S], bf16, tag="qp", name=f"qp{b}1")
            for m in range(2):
                for t in range(NJ):
                    acc = accp.tile([128, 512], f32, tag="acc", name="a1acc")
                    for e in range(EC):
                        qt, ready = quads[e // 4]
                        while sched.pe < ready:
                            yield NOTREADY, ready
                        nc.tensor.matmul(
                            acc[:], lhsT=wq_sb[:, e, m * 128:(m + 1) * 128],
                            rhs=qt[:, e % 4, t * 512:(t + 1) * 512],
                            start=(e == 0), stop=(e == EC - 1))
                        yield None, 0
                    nc.vector.tensor_copy(
                        qp_sb[(b, m)][:, t * 512:(t + 1) * 512], acc[:])

        def load_kv_ti(b, ti):
            quads = []
            for g in range(2):
                kt_in = kin.tile([128, 8, 512], bf16, tag="ki",
                                 name=f"k{b}_{ti}_{g}")
                nc.sync.dma_start(
                    kt_in[:],
                    kT_r[b, :, 8 * g:8 * g + 8, ti * 512:(ti + 1) * 512])
                sched.dma_issue(128 * 8 * 512 * 2)
                vt_in = vin.tile([128, 8, 512], bf16, tag="vi",
                                 name=f"v{b}_{ti}_{g}")
                nc.sync.dma_start(
                    vt_in[:],
                    vT_r[b, :, 8 * g:8 * g + 8, ti * 512:(ti + 1) * 512])
                quads.append((kt_in, vt_in,
                              sched.dma_issue(128 * 8 * 512 * 2)))
            return quads

        def a2_stream(b):
            """K.T / V projections, col block (ti) at a time, with its own
            prefetched input DMAs and V transposes."""
            kt2 = ktp.tile([128, S], bf16, tag="kt2", name=f"kt2_{b}")
            kt2_sb[b] = kt2
            pending = load_kv_ti(b, 0)
            for ti in range(NJ):
                kq = pending
                pending = load_kv_ti(b, ti + 1) if ti + 1 < NJ else None
                acc = accp.tile([128, 512], f32, tag="acc", name="a2acc")
                for e in range(EC):
                    kt_in, vt_in, ready = kq[e // 8]
                    while sched.pe < ready:
                        yield NOTREADY, ready
                    nc.tensor.matmul(
                        acc[0:64, :], lhsT=wk_sb[:, e, :],
                        rhs=kt_in[:, e % 8, :],
                        start=(e == 0), stop=(e == EC - 1))
                    yield None, 0
                    nc.tensor.matmul(
                        acc[64:128, :], lhsT=wv_sb[:, e, :],
                        rhs=vt_in[:, e % 8, :],
                        start=(e == 0), stop=(e == EC - 1),
                        tile_position=(0, 64))
                    yield None, 0
                nc.vector.tensor_copy(
                    kt2[0:64, ti * 512:(ti + 1) * 512], acc[0:64, :])
                # duplicate K.T into partitions 64:128 per-slice so the
                # last write to kt2 lands right after the last eviction
                nc.sync.dma_start(kt2[64:128, ti * 512:(ti + 1) * 512],
                                  kt2[0:64, ti * 512:(ti + 1) * 512])
                vt = vtp.tile([64, 512], bf16, tag="vt", name=f"vt{b}_{ti}")
                nc.vector.tensor_copy(vt[:], acc[64:128, :])
                for c in range(4):
                    nc.sync.dma_start_transpose(
                        out=vn_tiles[b][ti * 4 + c][:, 0:HD],
                        in_=vt[:, c * 128:(c + 1) * 128])

        def c_stream(b, j):
            """Phase C for output rows [j*512, (j+1)*512): 4 row-tiles."""
            for tt in range(4):
                t = j * 4 + tt
                ost = ostp.tile([128, E], bf16, tag="ost", name=f"o{b}_{t}")
                for e in range(4):
                    acc = accp.tile([128, 512], f32, tag="acc", name="cacc")
                    for kc in range(2):
                        nc.tensor.matmul(
                            acc[:],
                            lhsT=ctxT[(b, kc, j)][:, tt * 128:(tt + 1) * 128],
                            rhs=wo_sb[:, kc, e * 512:(e + 1) * 512],
                            start=(kc == 0), stop=(kc == 1))
                        yield None, 0
                    nc.vector.tensor_copy(ost[:, e * 512:(e + 1) * 512],
                                          acc[:])
                nc.sync.dma_start(out_d[b, t * 128:(t + 1) * 128, :], ost[:])
                sched.dma_issue(128 * E * 2)

        # ---- filler machinery ----
        streams = deque()

        def pump(n):
            done = tries = 0
            while done < n and streams and tries < len(streams):
                kind, ready = next(streams[0], ("END", 0))
                if kind is NOTREADY:
                    streams.rotate(-1)
                    tries += 1
                elif kind == "END":
                    streams.popleft()
                else:
                    done += 1
                    tries = 0

        def drain():
            while streams:
                stall = []
                progressed = False
                for _ in range(len(streams)):
                    kind, ready = next(streams[0], ("END", 0))
                    if kind is NOTREADY:
                        stall.append(ready)
                        streams.rotate(-1)
                    elif kind == "END":
                        streams.popleft()
                        progressed = True
                        break
                    else:
                        sched.pe += 213
                        progressed = True
                        break
                if not progressed and stall:
                    sched.pe = max(sched.pe, min(stall))

        def b_unit(b, j, h):
            """Attention for q-head h on q rows [j*512, (j+1)*512)."""
            kc, off = h // 2, (h % 2) * 64
            kt2 = kt2_sb[b]
            qp = qp_sb[(b, kc)]
            ctx_ps = cxp.tile([128, 512], f32, tag="ctx", name=f"cx{b}{j}{h}")
            pend = deque()

            def ctx_pair(p, pt):
                for half in range(2):
                    kv = 2 * p + half
                    nc.tensor.matmul(
                        ctx_ps[0:HD + 1, :],
                        lhsT=vn_tiles[b][kv][:, 0:HD + 1],
                        rhs=pt[:, half * 512:(half + 1) * 512],
                        start=(kv == 0), stop=(kv == NKV - 1))

            for p in range(NP):
                sc = scp.tile([128, 1024], f32, tag="sc", name="sc")
                for half in range(2):
                    kv = 2 * p + half
                    nc.tensor.matmul(
                        sc[:, half * 512:(half + 1) * 512],
                        lhsT=kt2[off:off + 64, kv * 128:(kv + 1) * 128],
                        rhs=qp[off:off + 64, j * 512:(j + 1) * 512],
                        start=True, stop=True)
                pt = ptp.tile([128, 1024], bf16, tag="pt", name="pt")
                nc.scalar.activation(pt[:], sc[:], EXP)
                pend.append((p, pt))
                if len(pend) > 2:
                    ctx_pair(*pend.popleft())
                sched.pe += 1222
                pump(2)
            while pend:
                ctx_pair(*pend.popleft())
            # normalize by softmax denominator (row HD)
            recip = smp.tile([1, 512], f32, tag="recip", name="recip")
            nc.vector.reciprocal(recip[:], ctx_ps[HD:HD + 1, :])
            rb = smp.tile([64, 512], f32, tag="rb", name="rb")
            nc.gpsimd.partition_broadcast(rb[:], recip[:])
            nc.vector.tensor_mul(
                ctxT[(b, kc, j)][off:off + 64, :], ctx_ps[0:64, :], rb[:])
            sched.pe += 2200
            pump(4)

        # ================= schedule =================
        # prolog: batch 0 A phase (DMA-paced)
        streams.append(a1_stream(0, q0))
        streams.append(a2_stream(0))
        drain()
        nc.sync.dma_start(wo_sb[:], wo_d.rearrange("(c p) e -> p c e", p=128))
        sched.dma_issue(QD * E * 2)
        sched.pe = max(sched.pe, sched.dma)

        carry_c = []
        for b in range(B):
            for kc in range(2):
                for j in range(NJ):
                    ctxT[(b, kc, j)] = ctp.tile(
                        [128, 512], bf16, name=f"ctxT{b}_{kc}_{j}")
            if b + 1 < B:
                qn = load_q(b + 1)
                streams.append(a1_stream(b + 1, qn))
                streams.append(a2_stream(b + 1))
            for g in carry_c:
                streams.append(g)
            carry_c = []
            for j in range(NJ):
                for h in range(HPC):
                    b_unit(b, j, h)
                # later j-segments of this batch's phase C fill the NEXT
                # batch's B window (this batch's window is already fed by
                # A of b+1); the last batch keeps its own C.
                if b + 1 < B and j >= 2:
                    carry_c.append(c_stream(b, j))
                else:
                    streams.append(c_stream(b, j))
        drain()

    nc.compile()
    return nc


def _get_nc():
    if "nc" not in _cache:
        _cache["nc"] = _build()
    return _cache["nc"]


def kernel(query, key, value, Wq, Wk, Wv, Wo, _trace=False):
    from concourse.bass_utils import run_bass_kernel_spmd

    def t_bf16(x):
        return np.ascontiguousarray(
            np.asarray(x, np.float32).astype(BF16).transpose(0, 2, 1))

    qT = t_bf16(query)
    kT = t_bf16(key)
    vT = t_bf16(value)
    # prescale Wq by the 1/sqrt(HD) attention scale (exact power of 2)
    Wq = (np.asarray(Wq, np.float32) * 0.125).astype(BF16)
    Wk = np.asarray(Wk, np.float32).astype(BF16)
    Wv = np.asarray(Wv, np.float32).astype(BF16)
    Wo = np.asarray(Wo, np.float32).astype(BF16)

    in_maps = []
    for c in range(NCORES):
        in_maps.append({
            "qT": qT, "kT": kT, "vT": vT,
            "wq": np.ascontiguousarray(Wq[:, c * QD:(c + 1) * QD]),
            "wk": np.ascontiguousarray(Wk[:, c * HD:(c + 1) * HD]),
            "wv": np.ascontiguousarray(Wv[:, c * HD:(c + 1) * HD]),
            "wo": np.ascontiguousarray(Wo[c * QD:(c + 1) * QD, :]),
        })

    nc = _get_nc()
    res = run_bass_kernel_spmd(nc, in_maps, list(range(NCORES)), trace=_trace)
    out = res.results[0]["out"].astype(np.float32)
    for c in range(1, NCORES):
        out += res.results[c]["out"].astype(np.float32)
    if _trace:
        _cache["last_exec_time_ns"] = res.exec_time_ns
        _cache["last_results"] = res
    return out


# revision 29
# speedup vs baseline: 1.1084x; 1.0133x over previous
"""GQA kernel for 8 trn2 NeuronCores.

Sharding: tensor-parallel over heads. Core c owns KV head c and Q heads
4c..4c+3 (q-dim cols 256c:256c+256 of Wq, col 64c:64c+64 of Wk/Wv, rows
256c:256c+256 of Wo). Each core computes a partial output [B,S,E]
(its ctx slice @ its Wo row-slice); host sums the 8 partials.

Device algorithm (per core) — v4, software-pipelined across batches:
  A1. Q.T = (Wq/8).T @ X.T as two head-PAIR tiles [128, S] (Wq prescaled
      on host so the PSUM->SBUF eviction is a plain copy).
  A2. K.T [64, S] (dup to 128 partitions) and V natural [S, 64+ones]
      via per-512-col accumulation groups (K rows 0:64, V rows 64:128
      col-packed PSUM) + DMA transposes for V.
  B.  16 units per batch = (q-head h, 512-wide q block j). Per unit,
      a kv-PAIR pipeline (8 steps): two score matmuls into a [128,1024]
      PSUM tile (2 banks), one exp on ScalarE -> pt bf16 [128,1024],
      two ctx matmuls (lagged one pair) accumulating ctx.T[0:65]
      (row 64 = softmax denominator, via ones column in V_aug).
      ScalarE paces the loop; the PE deficit is filled with matmuls from
      A1/A2 of the next batch and phase C of the current batch, pumped
      from filler streams gated by an emission-time DMA-arrival model
      (a filler that would wait on DMA would head-of-line block the
      in-order PE queue).
      Normalize: DVE recip + gpsimd partition-broadcast + DVE mul into
      ctxT tiles [128, 512] per (head-pair, j).
  C.  out_partial[t*128:+128, :] = ctxT.T @ Wo_c, 2-chunk accumulation,
      evicted bf16 and DMA'd out per 128-row tile.

All matmuls bf16 inputs / fp32 PSUM. PSUM banks: sc(2x2) + ctx(2) +
acc(2).
"""

import numpy as np
import ml_dtypes
from collections import deque

B = 2
S = 2048
E = 2048
HD = 64          # head dim
HPC = 4          # q heads per core
QD = HPC * HD    # 256 per-core q dims
NCORES = 8
EC = E // 128    # 16 contraction chunks
NKV = S // 128   # 16 kv chunks of 128
NP = NKV // 2    # 8 kv pairs
NJ = S // 512    # 4 q blocks of 512
BF16 = ml_dtypes.bfloat16

NOTREADY = object()

_cache = {}


def _build():
    from contextlib import ExitStack
    from concourse import bacc, tile
    import concourse.mybir as mybir

    bf16 = mybir.dt.bfloat16
    f32 = mybir.dt.float32
    EXP = mybir.ActivationFunctionType.Exp

    nc = bacc.Bacc(
        "TRN2", target_bir_lowering=False, debug=False, num_devices=NCORES)
    qT_d = nc.declare_dram_parameter("qT", [B, E, S], bf16, isOutput=False)
    kT_d = nc.declare_dram_parameter("kT", [B, E, S], bf16, isOutput=False)
    vT_d = nc.declare_dram_parameter("vT", [B, E, S], bf16, isOutput=False)
    wq_d = nc.declare_dram_parameter("wq", [E, QD], bf16, isOutput=False)
    wk_d = nc.declare_dram_parameter("wk", [E, HD], bf16, isOutput=False)
    wv_d = nc.declare_dram_parameter("wv", [E, HD], bf16, isOutput=False)
    wo_d = nc.declare_dram_parameter("wo", [QD, E], bf16, isOutput=False)
    out_d = nc.declare_dram_parameter("out", [B, S, E], bf16, isOutput=True)

    qT_r = qT_d.rearrange("b (c p) s -> b p c s", p=128)   # [B,128,16,S]
    kT_r = kT_d.rearrange("b (c p) s -> b p c s", p=128)
    vT_r = vT_d.rearrange("b (c p) s -> b p c s", p=128)

    class Sched:
        """Emission-time clock model: pe = estimated wall when the
        instruction being emitted will run; dma = when the DMA queue
        drains. Used only to gate filler emission, not for correctness."""
        NS_PER_BYTE = 1.0 / 360.0e9 * 1e9   # single shared DMA bus

        def __init__(self):
            self.pe = 0.0
            self.dma = 0.0

        MARGIN = 2500.0   # gate slack: bus queueing the model can't see

        def dma_issue(self, nbytes):
            self.dma = (max(self.dma, self.pe) + 1050
                        + nbytes * self.NS_PER_BYTE)
            return self.dma + self.MARGIN

    sched = Sched()

    with ExitStack() as ctx:
        tc = ctx.enter_context(tile.TileContext(nc))
        # ---- pools ----
        wpool = ctx.enter_context(tc.tile_pool(name="w", bufs=1))
        qin = ctx.enter_context(tc.tile_pool(name="qin", bufs=3))
        qsm = ctx.enter_context(tc.tile_pool(name="qsm", bufs=2))
        kin = ctx.enter_context(tc.tile_pool(name="kin", bufs=3))
        vin = ctx.enter_context(tc.tile_pool(name="vin", bufs=3))
        qts = ctx.enter_context(tc.tile_pool(name="qts", bufs=4))
        ktp = ctx.enter_context(tc.tile_pool(name="ktp", bufs=2))
        vtp = ctx.enter_context(tc.tile_pool(name="vtp", bufs=2))
        vnp = ctx.enter_context(tc.tile_pool(name="vnp", bufs=1))
        ptp = ctx.enter_context(tc.tile_pool(name="ptp", bufs=3))
        ctp = ctx.enter_context(tc.tile_pool(name="ctp", bufs=1))
        ostp = ctx.enter_context(tc.tile_pool(name="ostp", bufs=2))
        smp = ctx.enter_context(tc.tile_pool(name="smp", bufs=2))
        scp = ctx.enter_context(tc.tile_pool(name="scp", bufs=2, space="PSUM"))
        cxp = ctx.enter_context(tc.tile_pool(name="cxp", bufs=2, space="PSUM"))
        accp = ctx.enter_context(tc.tile_pool(name="accp", bufs=2,
                                              space="PSUM"))

        # ---- weights: wq first, wo deferred (only phase C needs it) ----
        wq_sb = wpool.tile([128, EC, QD], bf16)
        nc.sync.dma_start(wq_sb[:], wq_d.rearrange("(c p) m -> p c m", p=128))
        sched.dma_issue(E * QD * 2)

        def load_q(b):
            """Load X.T in pieces (2+2+4+4+4 e-chunks; the small first
            pieces let A1 start early). Returns per-e-chunk views."""
            chunks = []
            off = 0
            for n in (2, 2, 4, 4, 4):
                pool = qsm if n == 2 else qin
                qt = pool.tile([128, n, S], bf16, tag=f"qt{n}",
                               name=f"q{b}_{off}")
                nc.sync.dma_start(qt[:], qT_r[b, :, off:off + n, :])
                ready = sched.dma_issue(128 * n * S * 2)
                for i in range(n):
                    chunks.append((qt, i, ready))
                off += n
            return chunks

        q0 = load_q(0)

        wk_sb = wpool.tile([128, EC, HD], bf16)
        nc.sync.dma_start(wk_sb[:], wk_d.rearrange("(c p) m -> p c m", p=128))
        sched.dma_issue(E * HD * 2)
        wv_sb = wpool.tile([128, EC, HD], bf16)
        nc.sync.dma_start(wv_sb[:], wv_d.rearrange("(c p) m -> p c m", p=128))
        sched.dma_issue(E * HD * 2)
        wo_sb = wpool.tile([128, 2, E], bf16)

        ones64 = wpool.tile([1, 64], bf16)
        nc.vector.memset(ones64[:], 1.0)

        # V_aug tiles, ones column set once (transposes only write 0:HD)
        vn_tiles = [[vnp.tile([128, HD + 1], bf16, name=f"vn{b}_{c}")
                     for c in range(NKV)] for b in range(B)]
        for b in range(B):
            for c in range(NKV):
                nc.vector.memset(vn_tiles[b][c][:, HD:HD + 1], 1.0)

        # per-batch persistent tiles
        qp_sb = {}    # (b, pair) -> Q.T pair tile [128, S]
        kt2_sb = {}   # b -> K.T dup tile [128, S]
        ctxT = {}     # (b, pair, j) -> normalized ctx.T tile [128, 512]

        def a1_stream(b, quads):
            """Q.T projection: 8 groups of 16 accumulating matmuls."""
            qp_sb[(b, 0)] = qts.tile([128, S], bf16, tag="qp", name=f"qp{b}0")
            qp_sb[(b, 1)] = qts.tile([128, S], bf16, tag="qp", name=f"qp{b}1")
            for m in range(2):
                for t in range(NJ):
                    acc = accp.tile([128, 512], f32, tag="acc", name="a1acc")
                    for e in range(EC):
                        qt, i, ready = quads[e]
                        while sched.pe < ready:
                            yield NOTREADY, ready
                        nc.tensor.matmul(
                            acc[:], lhsT=wq_sb[:, e, m * 128:(m + 1) * 128],
                            rhs=qt[:, i, t * 512:(t + 1) * 512],
                            start=(e == 0), stop=(e == EC - 1))
                        yield None, 0
                    nc.vector.tensor_copy(
                        qp_sb[(b, m)][:, t * 512:(t + 1) * 512], acc[:])

        def load_kv_ti(b, ti):
            quads = []
            for g in range(2):
                kt_in = kin.tile([128, 8, 512], bf16, tag="ki",
                                 name=f"k{b}_{ti}_{g}")
                nc.sync.dma_start(
                    kt_in[:],
                    kT_r[b, :, 8 * g:8 * g + 8, ti * 512:(ti + 1) * 512])
                sched.dma_issue(128 * 8 * 512 * 2)
                vt_in = vin.tile([128, 8, 512], bf16, tag="vi",
                                 name=f"v{b}_{ti}_{g}")
                nc.sync.dma_start(
                    vt_in[:],
                    vT_r[b, :, 8 * g:8 * g + 8, ti * 512:(ti + 1) * 512])
                quads.append((kt_in, vt_in,
                              sched.dma_issue(128 * 8 * 512 * 2)))
            return quads

        def a2_stream(b):
            """K.T / V projections, col block (ti) at a time, with its own
            prefetched input DMAs and V transposes."""
            kt2 = ktp.tile([128, S], bf16, tag="kt2", name=f"kt2_{b}")
            kt2_sb[b] = kt2
            pending = load_kv_ti(b, 0)
            for ti in range(NJ):
                kq = pending
                pending = load_kv_ti(b, ti + 1) if ti + 1 < NJ else None
                acc = accp.tile([128, 512], f32, tag="acc", name="a2acc")
                for e in range(EC):
                    kt_in, vt_in, ready = kq[e // 8]
                    while sched.pe < ready:
                        yield NOTREADY, ready
                    nc.tensor.matmul(
                        acc[0:64, :], lhsT=wk_sb[:, e, :],
                        rhs=kt_in[:, e % 8, :],
                        start=(e == 0), stop=(e == EC - 1))
                    yield None, 0
                    nc.tensor.matmul(
                        acc[64:128, :], lhsT=wv_sb[:, e, :],
                        rhs=vt_in[:, e % 8, :],
                        start=(e == 0), stop=(e == EC - 1),
                        tile_position=(0, 64))
                    yield None, 0
                nc.vector.tensor_copy(
                    kt2[0:64, ti * 512:(ti + 1) * 512], acc[0:64, :])
                # duplicate K.T into partitions 64:128 per-slice so the
                # last write to kt2 lands right after the last eviction
                nc.sync.dma_start(kt2[64:128, ti * 512:(ti + 1) * 512],
                                  kt2[0:64, ti * 512:(ti + 1) * 512])
                vt = vtp.tile([64, 512], bf16, tag="vt", name=f"vt{b}_{ti}")
                nc.vector.tensor_copy(vt[:], acc[64:128, :])
                for c in range(4):
                    nc.sync.dma_start_transpose(
                        out=vn_tiles[b][ti * 4 + c][:, 0:HD],
                        in_=vt[:, c * 128:(c + 1) * 128])

        def c_stream(b, j):
            """Phase C for output rows [j*512, (j+1)*512): 4 row-tiles."""
            for tt in range(4):
                t = j * 4 + tt
                ost = ostp.tile([128, E], bf16, tag="ost", name=f"o{b}_{t}")
                for e in range(4):
                    acc = accp.tile([128, 512], f32, tag="acc", name="cacc")
                    for kc in range(2):
                        nc.tensor.matmul(
                            acc[:],
                            lhsT=ctxT[(b, kc, j)][:, tt * 128:(tt + 1) * 128],
                            rhs=wo_sb[:, kc, e * 512:(e + 1) * 512],
                            start=(kc == 0), stop=(kc == 1))
                        yield None, 0
                    nc.vector.tensor_copy(ost[:, e * 512:(e + 1) * 512],
                                          acc[:])
                nc.sync.dma_start(out_d[b, t * 128:(t + 1) * 128, :], ost[:])
                sched.dma_issue(128 * E * 2)

        # ---- filler machinery ----
        streams = deque()

        def pump(n):
            done = tries = 0
            while done < n and streams and tries < len(streams):
                kind, ready = next(streams[0], ("END", 0))
                if kind is NOTREADY:
                    streams.rotate(-1)
                    tries += 1
                elif kind == "END":
                    streams.popleft()
                else:
                    done += 1
                    tries = 0

        def drain():
            while streams:
                stall = []
                progressed = False
                for _ in range(len(streams)):
                    kind, ready = next(streams[0], ("END", 0))
                    if kind is NOTREADY:
                        stall.append(ready)
                        streams.rotate(-1)
                    elif kind == "END":
                        streams.popleft()
                        progressed = True
                        break
                    else:
                        sched.pe += 213
                        progressed = True
                        break
                if not progressed and stall:
                    sched.pe = max(sched.pe, min(stall))

        def b_unit(b, j, h):
            """Attention for q-head h on q rows [j*512, (j+1)*512)."""
            kc, off = h // 2, (h % 2) * 64
            kt2 = kt2_sb[b]
            qp = qp_sb[(b, kc)]
            ctx_ps = cxp.tile([128, 512], f32, tag="ctx", name=f"cx{b}{j}{h}")
            pend = deque()

            def ctx_pair(p, pt):
                for half in range(2):
                    kv = 2 * p + half
                    nc.tensor.matmul(
                        ctx_ps[0:HD + 1, :],
                        lhsT=vn_tiles[b][kv][:, 0:HD + 1],
                        rhs=pt[:, half * 512:(half + 1) * 512],
                        start=(kv == 0), stop=(kv == NKV - 1))

            for p in range(NP):
                sc = scp.tile([128, 1024], f32, tag="sc", name="sc")
                for half in range(2):
                    kv = 2 * p + half
                    nc.tensor.matmul(
                        sc[:, half * 512:(half + 1) * 512],
                        lhsT=kt2[off:off + 64, kv * 128:(kv + 1) * 128],
                        rhs=qp[off:off + 64, j * 512:(j + 1) * 512],
                        start=True, stop=True)
                pt = ptp.tile([128, 1024], bf16, tag="pt", name="pt")
                nc.scalar.activation(pt[:], sc[:], EXP)
                pend.append((p, pt))
                if len(pend) > 2:
                    ctx_pair(*pend.popleft())
                sched.pe += 1222
                pump(2)
            while pend:
                ctx_pair(*pend.popleft())
            # normalize by softmax denominator (row HD)
            recip = smp.tile([1, 512], f32, tag="recip", name="recip")
            nc.vector.reciprocal(recip[:], ctx_ps[HD:HD + 1, :])
            rb = smp.tile([64, 512], f32, tag="rb", name="rb")
            nc.gpsimd.partition_broadcast(rb[:], recip[:])
            nc.vector.tensor_mul(
                ctxT[(b, kc, j)][off:off + 64, :], ctx_ps[0:64, :], rb[:])
            sched.pe += 2200
            pump(4)

        # ================= schedule =================
        # prolog: batch 0 A phase (DMA-paced)
        streams.append(a1_stream(0, q0))
        streams.append(a2_stream(0))
        drain()
        nc.sync.dma_start(wo_sb[:], wo_d.rearrange("(c p) e -> p c e", p=128))
        sched.dma_issue(QD * E * 2)
        sched.pe = max(sched.pe, sched.dma)

        carry_c = []
        for b in range(B):
            for kc in range(2):
                for j in range(NJ):
                    ctxT[(b, kc, j)] = ctp.tile(
                        [128, 512], bf16, name=f"ctxT{b}_{kc}_{j}")
            if b + 1 < B:
                qn = load_q(b + 1)
                streams.append(a1_stream(b + 1, qn))
                streams.append(a2_stream(b + 1))
            for g in carry_c:
                streams.append(g)
            carry_c = []
            for j in range(NJ):
                for h in range(HPC):
                    b_unit(b, j, h)
                # later j-segments of this batch's phase C fill the NEXT
                # batch's B window (this batch's window is already fed by
                # A of b+1); the last batch keeps its own C.
                if b + 1 < B and j >= 2:
                    carry_c.append(c_stream(b, j))
                else:
                    streams.append(c_stream(b, j))
        drain()

    nc.compile()
    return nc


def _get_nc():
    if "nc" not in _cache:
        _cache["nc"] = _build()
    return _cache["nc"]


def kernel(query, key, value, Wq, Wk, Wv, Wo, _trace=False):
    from concourse.bass_utils import run_bass_kernel_spmd

    def t_bf16(x):
        return np.ascontiguousarray(
            np.asarray(x, np.float32).astype(BF16).transpose(0, 2, 1))

    qT = t_bf16(query)
    kT = t_bf16(key)
    vT = t_bf16(value)
    # prescale Wq by the 1/sqrt(HD) attention scale (exact power of 2)
    Wq = (np.asarray(Wq, np.float32) * 0.125).astype(BF16)
    Wk = np.asarray(Wk, np.float32).astype(BF16)
    Wv = np.asarray(Wv, np.float32).astype(BF16)
    Wo = np.asarray(Wo, np.float32).astype(BF16)

    in_maps = []
    for c in range(NCORES):
        in_maps.append({
            "qT": qT, "kT": kT, "vT": vT,
            "wq": np.ascontiguousarray(Wq[:, c * QD:(c + 1) * QD]),
            "wk": np.ascontiguousarray(Wk[:, c * HD:(c + 1) * HD]),
            "wv": np.ascontiguousarray(Wv[:, c * HD:(c + 1) * HD]),
            "wo": np.ascontiguousarray(Wo[c * QD:(c + 1) * QD, :]),
        })

    nc = _get_nc()
    res = run_bass_kernel_spmd(nc, in_maps, list(range(NCORES)), trace=_trace)
    out = res.results[0]["out"].astype(np.float32)
    for c in range(1, NCORES):
        out += res.results[c]["out"].astype(np.float32)
    if _trace:
        _cache["last_exec_time_ns"] = res.exec_time_ns
        _cache["last_results"] = res
    return out
